# revision 93
# baseline (speedup 1.0000x reference)
"""Trainium2 Bass kernel for nn_DynamicBlock (sparse-token attention + MLP block).

Contract: kernel(**inputs) takes the FULL unsharded inputs (as produced by
reference.setup_inputs()) and returns the FULL [B, T, D] output.

Sharding (pairwise tensor-parallel): 8 cores = 4 batches x 2 halves.
Each core of a batch pair:
 - K/V projections (+rope on K) over all T for its 4 kv-heads, interleaved
   with the first attention pass to hide the hidden-state DMA stream,
 - Q proj + rope for its 8 q-heads over ALL 512 selected queries,
 - causal attention (its heads, all 512 queries) one 256-query half at a
   time; after each half: o-proj partial over its heads' o_w columns and a
   2-rank bf16 AllReduce of that half's partial attn_out (AR of half A
   overlaps the attention of half B; AR of B overlaps the MLP on A),
 - h = AR-sum + residual, rmsnorm2, then MLP over its d_ff HALF (16 of 32
   ff-chunks) for ALL 512 tokens, emitting the partial gated update
   Z_r = selg_r + g*h*alpha_r + g*mlp_r (alpha = 1 on rank 0, 0 on rank 1),
 - host sums Z_0 + Z_1 per pair and scatters into hidden_states.

MLP runs in fp8e4 (DoubleRow perf mode, 2x PE throughput): gate/up/down
weights are pre-scaled powers of two chosen to keep fp8 operands in normal
range, activations (n2, act) are quantized on the fly, and the combined
scale is folded into the host-side gating multiplier. MLP weights are
DMA'd once into SBUF (3 large transfers issued at kernel start, landing
during the attention phase) and reused for both query halves.

Softmax normalization uses the scalar engine's Reciprocal activation read
directly from the PSUM row-sum row (partition 64, a legal PE base) plus a
1xN bf16 broadcast matmul; rmsnorm2 uses Rsqrt the same way. Both avoid
the slow DVE reciprocal and SBUF->SBUF descriptor hops.

Everything on-device runs in a transposed layout ([feature, token]);
rotate_half for rope is a PE matmul with a signed permutation matrix.
"""

import sys

sys.path.insert(0, "/opt/trn_rl_repo")

import numpy as np
import ml_dtypes

import concourse.bass as bass
import concourse.tile as tile
from concourse import mybir
from concourse import bass_utils as _bu
from concourse.bass_utils import run_bass_kernel_spmd
from concourse.vector_clock import ScopedClock, VectorClock



BF16 = mybir.dt.bfloat16
F32 = mybir.dt.float32
FP8 = mybir.dt.float8e4
AF = mybir.ActivationFunctionType
OP = mybir.AluOpType
PM = mybir.MatmulPerfMode

B, T, D = 4, 2048, 1024
H, KV, HD = 16, 8, 64
DFF = 4096
KSEL = 512
EPS = 1e-6

NQ = 512          # selected queries per batch (all of them, head-split)
NQH = 256         # query half processed per attention pass
ND = D // 128     # 8 d-tiles
NT = T // 128     # 16 key tiles
HL = H // 2       # 8 local q heads
KVL = KV // 2     # 4 local kv heads
NKC = KVL * HD // 128  # 2 local k-output chunks (2 kv heads each)
NQC = HL * HD // 128   # 4 local q-output chunks (2 q heads each)
NFC = DFF // 128       # 32 ff chunks
NFL = NFC // 2         # 16 local ff chunks (d_ff tensor-parallel)
NCORES = 8
PAIRS = [[0, 1], [2, 3], [4, 5], [6, 7]]

# fp8 weight scales (powers of two; folded into host-side gate multiplier)
S_G = 512.0
S_U = 16.0
S_D = 512.0

# local q-head layout: q-chunk 2c holds local heads (4c, 4c+2) on partition
# halves (local kv heads 2c / 2c+1), chunk 2c+1 holds (4c+1, 4c+3).
TILE_HEADS_L = []
for c in range(2):
    TILE_HEADS_L.append((4 * c, 4 * c + 2))
    TILE_HEADS_L.append((4 * c + 1, 4 * c + 3))
HEAD_PERM_L = np.array(
    [h * HD + i for pair in TILE_HEADS_L for h in pair for i in range(HD)])


# ---------------------------------------------------------------------------
# walrus workarounds: this toolchain encodes at most ONE semaphore wait per
# instruction. Split the tile tail-drain into per-proc drains and move excess
# waits onto NoOps.
# ---------------------------------------------------------------------------

def _patched_drain_and_barrier(self, tick_clock, wait_clock):
    gc = tick_clock.global_clock
    n = len(gc)
    for i in range(n):
        t = gc[i]
        if t > 0:
            vec = [0] * n
            vec[i] = t
            d = self.nc.sync.drain()
            wait_clock.add_sem_waits(d.ins, ScopedClock({None: VectorClock(vec)}))
    self.nc.all_engine_barrier()
    popped = self.nc._tile_sem_poison_stack.pop()
    assert popped is self._sem_poison
    self.nc.clear_and_free_semaphores(list(self.sems.allocated().values()))


tile.TileContext._drain_and_barrier = _patched_drain_and_barrier

_MAX_WAITS = 1


def _split_excess_waits(nc):
    for f in nc.m.functions:
        for bb in f.blocks:
            new = []
            for inst in bb.instructions:
                si = inst.sync_info
                if si is not None and si.on_wait is not None and len(si.on_wait) > _MAX_WAITS:
                    waits = list(si.on_wait)
                    excess, keep = waits[:-_MAX_WAITS], waits[-_MAX_WAITS:]
                    k = 0
                    while excess:
                        chunk, excess = excess[:_MAX_WAITS], excess[_MAX_WAITS:]
                        new.append(mybir.InstNoOp(
                            name=f"{inst.name}_ws{k}",
                            engine=inst.engine,
                            sync_info=mybir.SyncInfo(on_wait=chunk, on_update=[])))
                        k += 1
                    inst.sync_info = mybir.SyncInfo(
                        on_wait=keep, on_update=list(si.on_update or []))
                new.append(inst)
            bb.instructions = new


def act_unchecked(eng, out, in_, func, bias=0.0, scale=1.0):
    """scalar.activation without the Reciprocal/Rsqrt accuracy guard (our
    tolerance is 2e-2; the LUT error is ~1e-3)."""
    inputs = [eng.lower_ap(in_)]
    for arg in [bias, scale, 0.0]:
        if isinstance(arg, bass.AP):
            inputs.append(eng.lower_ap(arg))
        else:
            inputs.append(mybir.ImmediateValue(dtype=mybir.dt.float32, value=arg))
    outputs = [eng.lower_ap(out)]
    return eng.add_instruction(
        mybir.InstActivation(
            name=eng.bass.get_next_instruction_name(),
            func=func, ins=inputs, outs=outputs))


# ---------------------------------------------------------------------------
# device program
# ---------------------------------------------------------------------------

def build_program(qlo, qhi, dbg=False):
    """qlo/qhi: dict[(qh, tt)] compile-time query ranges within each 256-query
    half (uniform across cores/batches)."""
    nc = bass.Bass(trn_type="TRN2", target_bir_lowering=False, debug=False)

    def inp(name, shape, dt):
        return nc.dram_tensor(name, shape, dt, kind="ExternalInput").ap()

    # ALL inputs are host-pre-arranged partition-major so every DMA is a
    # linear copy (128 descriptors of 4-32KB); strided/transposing DMAs are
    # descriptor-rate bound (~8.5ns/descriptor) and 6-8x slower.
    xnC = inp("xnC", [4, 128, ND, 512], BF16)     # normalized hidden.T, chunked
    nselT = inp("nselT", [128, ND, NQ], BF16)     # host-normalized selected.T
    selO = inp("selO", [128, ND, NQ], BF16)       # raw selected rows.T
    qwT = inp("qwT", [128, ND, HL * HD], BF16)
    kwT = inp("kwT", [128, ND, KVL * HD], BF16)
    vwT = inp("vwT", [128, ND, KVL * HD], BF16)
    owT = inp("owT", [128, NQC, D], BF16)
    gw = inp("gw", [128, NFL, ND, 128], FP8)
    uw = inp("uw", [128, NFL, ND, 128], FP8)
    dw = inp("dw", [128, ND, NFL, 128], FP8)
    # packed small constants: f32 block and bf16 block (one DMA each)
    NCF = 4 + 2 + KVL * HD + NQ + NT + NQ + NQ
    NCB = 128 + NQ + NQ + 64 + 2 * T
    cstF = inp("cstF", [128, NCF], F32)
    cstB = inp("cstB", [128, NCB], BF16)
    selg = inp("selg", [128, ND, NQ], F32)  # selres*(1-g) on rank 0, zeros rank 1

    updT = nc.dram_tensor("updT", [128, ND, NQ], BF16,
                          kind="ExternalOutput").ap()

    with tile.TileContext(nc, pool_alloc_mode="queue") as tc:
        with tc.tile_pool(name="ps", bufs=8, space="PSUM") as ps, \
             tc.tile_pool(name="persist", bufs=1) as pp, \
             tc.tile_pool(name="rows", bufs=2) as rowp, \
             tc.tile_pool(name="dramp", bufs=1, space="DRAM") as dram:

            # AllGather of fp8 partials + local add beats AllReduce: the CC
            # cost is a fixed ~15us overhead plus bytes moved at ~30GB/s, and
            # AR pays a 1.875x protocol multiplier on top.
            cc_in = [dram.tile([128, ND, NQH], FP8, name=f"cc_in{i}")
                     for i in range(2)]
            cc_out = [dram.tile([2, 128, ND, NQH], FP8, name=f"cc_out{i}")
                      for i in range(2)]
            # tiny warmup gather issued at kernel start: absorbs any first-op
            # negotiation/ramp cost on the CC cores before the real gathers
            warm_in = dram.tile([128, 2], BF16, name="warm_in")
            warm_out = dram.tile([2, 128, 2], BF16, name="warm_out")
            nc.gpsimd.collective_compute(
                "AllGather", OP.bypass, replica_groups=PAIRS,
                ins=[warm_in.opt()], outs=[warm_out.opt()])

            # ---- persistent tiles ------------------------------------------
            hTt = pp.tile([128, ND, NQ], BF16, name="hTt")
            n2T = pp.tile([128, ND, NQ], FP8, name="n2T")
            ctxT = pp.tile([128, NQC, NQ], BF16, name="ctxT")
            actT = pp.tile([128, NFL, NQ], FP8, name="actT")
            ones_t = pp.tile([128, 1], BF16, name="ones_t")
            nc.vector.memset(ones_t, 1.0)
            eps_t = pp.tile([1, 1], F32, name="eps_t")
            nc.vector.memset(eps_t, EPS)
            ones65 = pp.tile([65, 128], BF16, name="ones65")
            nc.vector.memset(ones65, 1.0)

            cF = pp.tile([128, NCF], F32, name="cF")
            cB = pp.tile([128, NCB], BF16, name="cB")
            o_ = 0
            c_qb = cF[:, o_:o_ + NQC]; o_ += NQC
            c_kb = cF[:, o_:o_ + NKC]; o_ += NKC
            c_vb = cF[:, o_:o_ + KVL * HD]; o_ += KVL * HD
            c_pos = cF[:, o_:o_ + NQ]; o_ += NQ
            c_tv = cF[:, o_:o_ + NT]; o_ += NT
            c_g = cF[:, o_:o_ + NQ]; o_ += NQ
            c_gh = cF[:, o_:o_ + NQ]; o_ += NQ
            assert o_ == NCF
            c_rm = cB[:, 0:128]
            c_cq = cB[:, 128:128 + NQ]
            c_sq = cB[:, 128 + NQ:128 + 2 * NQ]
            ident64 = cB[:, 128 + 2 * NQ:128 + 2 * NQ + 64]  # I on parts 0:64
            _o2 = 128 + 2 * NQ + 64
            c_ck = cB[:, _o2:_o2 + T]
            c_sk = cB[:, _o2 + T:_o2 + 2 * T]
            pA_cm = tc.tile_pool(name="pA", bufs=1)
            pA = pA_cm.__enter__()
            kT = pA.tile([128, NKC, T], BF16, name="kT")
            vplus = pA.tile([128, NT, KVL, HD + 1], BF16, name="vplus")
            nc.vector.memset(vplus[:, :, :, HD:HD + 1], 1.0)
            qrT = pA.tile([128, NQC, NQ], BF16, name="qrT")
            w_o = pA.tile([128, NQC, D], BF16, name="w_o")
            selOs = pA.tile([128, ND, NQ], BF16, name="selOs")

            p4_cm = tc.tile_pool(name="ph4", bufs=1)
            p4 = p4_cm.__enter__()

            pN_cm = tc.tile_pool(name="pN", bufs=1)
            pN = pN_cm.__enter__()
            xn = pN.tile([128, 4, ND, 512], BF16, name="xn")
            w_k = pN.tile([128, ND, KVL * HD], BF16, name="w_k")
            w_v = pN.tile([128, ND, KVL * HD], BF16, name="w_v")
            w_q = pN.tile([128, ND, HL * HD], BF16, name="w_q")
            nsel = pN.tile([128, ND, NQ], BF16, name="nsel")

            # ---- input DMAs (all linear; issue order = priority per engine;
            # xn chunks spread over the three DMA queues) ---
            nc.gpsimd.dma_start(out=w_k[:, :, 0:128], in_=kwT[:, :, 0:128])
            nc.gpsimd.dma_start(out=w_k[:, :, 128:256], in_=kwT[:, :, 128:256])
            nc.gpsimd.dma_start(out=w_v, in_=vwT)
            nc.sync.dma_start(out=xn[:, 0, 0:ND // 2], in_=xnC[0][:, 0:ND // 2])
            nc.sync.dma_start(out=xn[:, 0, ND // 2:ND],
                              in_=xnC[0][:, ND // 2:ND])
            nc.sync.dma_start(out=xn[:, 3], in_=xnC[3])
            nc.scalar.dma_start(out=cF, in_=cstF)
            nc.scalar.dma_start(out=cB, in_=cstB)
            nc.scalar.dma_start(out=xn[:, 1], in_=xnC[1])
            nc.gpsimd.dma_start(out=nsel, in_=nselT)
            nc.gpsimd.dma_start(out=w_q, in_=qwT)
            nc.gpsimd.dma_start(out=xn[:, 2], in_=xnC[2])
            nc.gpsimd.dma_start(out=w_o, in_=owT)
            nc.gpsimd.dma_start(out=selOs, in_=selO)

            # ==================================================================
            # Phase 1: K (+rope) and V per 512-token chunk, interleaved with
            # the first attention t-loop to hide the xn DMA stream.
            # ==================================================================
            def kv_chunk(ch, p2):
                    cs = slice(ch * 512, (ch + 1) * 512)
                    for kc in range(NKC):
                        kps = ps.tile([128, 512], F32, name="kps", tag="ps")
                        for dt in range(ND):
                            nc.tensor.matmul(
                                kps, lhsT=w_k[:, dt, kc * 128:(kc + 1) * 128],
                                rhs=xn[:, ch, dt, :],
                                start=(dt == 0), stop=(dt == ND - 1))
                        kraw = p2.tile([128, 512], BF16, name="kraw")
                        nc.vector.tensor_scalar(
                            out=kraw, in0=kps, scalar1=c_kb[:, kc:kc + 1],
                            scalar2=None, op0=OP.add)
                        rot = ps.tile([128, 512], F32, name="rot", tag="ps")
                        nc.tensor.matmul(rot, lhsT=c_rm, rhs=kraw,
                                         start=True, stop=True)
                        dst = kT[:, kc, cs]
                        tmp = p2.tile([128, 512], BF16, name="tmp")
                        nc.vector.tensor_mul(out=tmp, in0=rot, in1=c_sk[:, cs])
                        nc.vector.tensor_mul(out=dst, in0=kraw, in1=c_ck[:, cs])
                        nc.vector.tensor_add(out=dst, in0=dst, in1=tmp)

                    for tt in range(ch * 4, ch * 4 + 4):
                        vps = ps.tile([128, 512], F32, name="vps", tag="ps")
                        to = (tt % 4) * 128
                        for dt in range(ND):
                            nc.tensor.matmul(
                                vps[:, 0:KVL * HD],
                                lhsT=xn[:, ch, dt, to:to + 128],
                                rhs=w_v[:, dt, :],
                                start=(dt == 0), stop=(dt == ND - 1))
                        nc.vector.tensor_add(
                            out=vplus[:, tt, :, 0:HD],
                            in0=vps[:, 0:KVL * HD].rearrange(
                                "p (h d) -> p h d", h=KVL),
                            in1=c_vb.rearrange("p (h d) -> p h d", h=KVL))

            # ==================================================================
            # Phase 2: Q proj + rope (host-normalized input)
            # ==================================================================
            def qproj(p3):
                for qc in range(NQC):
                    qps = ps.tile([128, 512], F32, name="qps", tag="ps")
                    for dt in range(ND):
                        nc.tensor.matmul(
                            qps[:, 0:NQ], lhsT=w_q[:, dt, qc * 128:(qc + 1) * 128],
                            rhs=nsel[:, dt, :],
                            start=(dt == 0), stop=(dt == ND - 1))
                    qraw = p3.tile([128, NQ], BF16, name="qraw")
                    nc.vector.tensor_scalar(
                        out=qraw, in0=qps[:, 0:NQ], scalar1=c_qb[:, qc:qc + 1],
                        scalar2=None, op0=OP.add)
                    rotq = ps.tile([128, 512], F32, name="rotq", tag="ps")
                    nc.tensor.matmul(rotq[:, 0:NQ], lhsT=c_rm, rhs=qraw,
                                     start=True, stop=True)
                    dst = qrT[:, qc, :]
                    tmpq = p3.tile([128, NQ], BF16, name="tmpq")
                    nc.vector.tensor_mul(out=tmpq, in0=rotq[:, 0:NQ], in1=c_sq)
                    nc.vector.tensor_mul(out=dst, in0=qraw, in1=c_cq)
                    nc.vector.tensor_add(out=dst, in0=dst, in1=tmpq)

            # ==================================================================
            # Phases 3-7 per query half: attention t-loop, eviction, o-proj +
            # AllReduce (overlapped), h + rmsnorm2, d_ff-split MLP + Z output.
            # ==================================================================

            def alloc_cps(qh):
                # one accumulator per (kc, kv-half); free dim packs BOTH
                # q-head (ab) blocks so the ctx matmul is a single 3D-out
                # instruction per (tt, kc, half)
                cps = {}
                for kc in range(NKC):
                    for hf in range(2):
                        cps[(kc, hf)] = ps.tile([128, 512], F32,
                                                name=f"cps{qh}{kc}{hf}",
                                                tag="ps")
                return cps

            def attn_tloop(qh, cps, tts=None, hooks=None):
                qs0 = qh * NQH
                live = [t_ for t_ in range(NT) if qlo[(qh, t_)] < NQH]
                last_tt = max(live)
                for ti, tt in enumerate(live if tts is None else tts):
                    if hooks is not None and ti in hooks:
                        hooks[ti]()
                    lo = qlo[(qh, tt)]
                    hi = qhi[(qh, tt)]
                    mask = None
                    if hi > lo:
                        mask = p4.tile([128, 512], BF16, name="mask", bufs=2)
                        for mh in range(2):
                            nc.vector.tensor_scalar(
                                out=mask[:, mh * NQH + lo:mh * NQH + hi],
                                in0=c_pos[:, qs0 + lo:qs0 + hi],
                                scalar1=c_tv[:, tt:tt + 1], scalar2=None,
                                op0=OP.is_ge)
                    # stage-major within the t-tile: all scores first, then
                    # all ctx matmuls — the exps pipeline on the scalar
                    # engine behind the remaining scores instead of
                    # serializing a score->exp->ctx ring per head group.
                    ptl = []
                    for kc in range(NKC):
                        for half in range(2):
                            hs_ = slice(half * 64, (half + 1) * 64)
                            sp = ps.tile([128, 512], F32, name="sp", tag="ps")
                            # one matmul streams both q-head chunks (3D rhs)
                            nc.tensor.matmul(
                                sp.rearrange("p (a q) -> p a q", a=2)[:, :, lo:NQH],
                                lhsT=kT[hs_, kc, tt * 128:(tt + 1) * 128],
                                rhs=qrT[hs_, 2 * kc:2 * kc + 2,
                                        qs0 + lo:qs0 + NQH],
                                start=True, stop=True)
                            pt = p4.tile([128, 2, NQH], BF16, name="pt", bufs=8)
                            nc.scalar.activation(
                                out=pt[:, :, lo:NQH],
                                in_=sp.rearrange("p (h q) -> p h q", h=2)[:, :, lo:NQH],
                                func=AF.Exp)
                            if mask is not None:
                                nc.vector.tensor_mul(
                                    out=pt[:, :, lo:hi],
                                    in0=pt[:, :, lo:hi],
                                    in1=mask.rearrange(
                                        "p (h q) -> p h q", h=2)[:, :, lo:hi])
                            ptl.append((kc, half, pt))
                    for kc, half, pt in ptl:
                        cp = cps[(kc, half)]
                        nc.tensor.matmul(
                            cp.rearrange("p (a q) -> p a q",
                                         a=2)[0:HD + 1, :, lo:NQH],
                            lhsT=vplus[:, tt, 2 * kc + half, 0:HD + 1],
                            rhs=pt[:, :, lo:NQH],
                            start=(tt == 0), stop=(tt == last_tt))

            def attn_evict(qh, cps):
                """ctx rows sit at PSUM partitions 0..63, the row-sum at
                partition 64 (a legal PE base). Reciprocal on the scalar
                engine straight from PSUM, 1x64 bf16 broadcast matmul, then
                per-head scaling. Second head of each pair staged and moved
                with a single SBUF->SBUF DMA."""
                qsl = slice(qh * NQH, qh * NQH + NQH)
                tiles = [(k_, h_) for k_ in range(NKC) for h_ in range(2)]
                # stage-major emission: all recips, then all broadcasts, ...
                # so the four tiles pipeline across engines instead of
                # serializing one ~1.7us chain per tile.
                rrs, rbs, casts, stages, psbs = [], [], {}, [], []
                for kc, hf in tiles:
                    cp = cps[(kc, hf)]
                    rr = p4.tile([65, 512], BF16, name="rr", bufs=4)
                    act_unchecked(nc.scalar, rr[64:65, :], cp[64:65, :],
                                  AF.Reciprocal)
                    rrs.append(rr)
                for g, (kc, hf) in enumerate(tiles):
                    rb = ps.tile([128, 512], F32, name="rb", tag="ps")
                    nc.tensor.matmul(rb[0:64, :], lhsT=ones65[64:65, 0:64],
                                     rhs=rrs[g][64:65, :],
                                     start=True, stop=True)
                    rbs.append(rb)
                for g, (kc, hf) in enumerate(tiles):
                    rb_sb = p4.tile([64, 512], BF16, name="rb_sb", bufs=4)
                    nc.vector.tensor_copy(out=rb_sb, in_=rbs[g][0:64, :])
                    casts[(kc, hf)] = rb_sb
                # output chunk 2kc+ab: head[0] from kv-half-0 tile's ab block,
                # head[1] from kv-half-1 tile's ab block
                chunks = [(k_, a_) for k_ in range(NKC) for a_ in range(2)]
                for kc, ab in chunks:
                    absl = slice(ab * NQH, (ab + 1) * NQH)
                    # second head of the pair staged for the PE identity move
                    # (SBUF->SBUF DMAs are descriptor-rate bound: ~17us)
                    stage = p4.tile([64, NQH], BF16, name="stage", bufs=4)
                    nc.vector.tensor_mul(
                        out=stage,
                        in0=cps[(kc, 1)][0:HD, absl],
                        in1=casts[(kc, 1)][:, absl])
                    stages.append(stage)
                    nc.vector.tensor_mul(
                        out=ctxT[0:64, 2 * kc + ab, qsl],
                        in0=cps[(kc, 0)][0:HD, absl],
                        in1=casts[(kc, 0)][:, absl])
                for g, (kc, ab) in enumerate(chunks):
                    psb = ps.tile([128, 512], F32, name="psb", tag="ps")
                    nc.tensor.matmul(psb[64:128, 0:NQH],
                                     lhsT=ident64[0:64, :], rhs=stages[g],
                                     start=True, stop=True)
                    psbs.append(psb)
                for g, (kc, ab) in enumerate(chunks):
                    nc.vector.tensor_copy(
                        out=ctxT[64:128, 2 * kc + ab, qsl],
                        in_=psbs[g][64:128, 0:NQH])

            def oproj(qh, p5):
                qsl = slice(qh * NQH, qh * NQH + NQH)
                o_st = p5.tile([128, ND, NQH], FP8, name="o_st")
                for dc in range(ND):
                    ops_ = ps.tile([128, 512], F32, name="ops_", tag="ps")
                    for hc in range(NQC):
                        nc.tensor.matmul(
                            ops_[:, 0:NQH],
                            lhsT=w_o[:, hc, dc * 128:(dc + 1) * 128],
                            rhs=ctxT[:, hc, qsl],
                            start=(hc == 0), stop=(hc == NQC - 1))
                    nc.vector.tensor_copy(out=o_st[:, dc, :], in_=ops_[:, 0:NQH])
                nc.sync.dma_start(out=cc_in[qh], in_=o_st)
                nc.gpsimd.collective_compute(
                    "AllGather", OP.bypass, replica_groups=PAIRS,
                    ins=[cc_in[qh].opt()], outs=[cc_out[qh].opt()])

            def hnorm(qh, p6, selg_s):
                """h = AR + residual for this half; rmsnorm2 -> n2T half (fp8);
                Z base ghs = selg + c_gh * h."""
                qsl = slice(qh * NQH, qh * NQH + NQH)
                hsb = p6.tile([128, 2, ND, NQH], FP8, name="hsb")
                nc.sync.dma_start(out=hsb[:, 0], in_=cc_out[qh][0])
                nc.sync.dma_start(out=hsb[:, 1], in_=cc_out[qh][1])
                ssn = ps.tile([128, 512], F32, name="ssn", tag="ps")
                for dt in range(ND):
                    eng = nc.vector if dt % 2 == 0 else nc.gpsimd
                    eng.tensor_add(out=hTt[:, dt, qsl],
                                   in0=hsb[:, 0, dt, :],
                                   in1=hsb[:, 1, dt, :])
                    eng.tensor_add(out=hTt[:, dt, qsl],
                                   in0=hTt[:, dt, qsl],
                                   in1=selOs[:, dt, qsl])
                    sq6 = p6.tile([128, NQH], BF16, name="sq6", bufs=4)
                    eng.tensor_mul(out=sq6, in0=hTt[:, dt, qsl],
                                   in1=hTt[:, dt, qsl])
                    nc.tensor.matmul(ssn[0:1, 0:NQH], lhsT=ones_t, rhs=sq6,
                                     start=(dt == 0), stop=(dt == ND - 1))
                rrow = rowp.tile([1, NQH], BF16, name="rrow", tag="row")
                act_unchecked(nc.scalar, rrow, ssn[0:1, 0:NQH], AF.Rsqrt,
                              bias=eps_t[0:1, 0:1], scale=1.0 / D)
                rbc = ps.tile([128, 512], F32, name="rbc", tag="ps")
                nc.tensor.matmul(rbc[:, 0:NQH], lhsT=ones65[0:1, :], rhs=rrow,
                                 start=True, stop=True)
                rbc_sb = p6.tile([128, NQH], BF16, name="rbc_sb")
                nc.vector.tensor_copy(out=rbc_sb, in_=rbc[:, 0:NQH])
                rbc_b4 = bass.AP(tensor=rbc_sb.tensor, offset=rbc_sb.offset,
                                 ap=[rbc_sb.ap[0], [0, ND // 2], rbc_sb.ap[1]])
                nc.vector.tensor_mul(out=n2T[:, 0:ND // 2, qsl],
                                     in0=hTt[:, 0:ND // 2, qsl], in1=rbc_b4)
                nc.gpsimd.tensor_mul(out=n2T[:, ND // 2:ND, qsl],
                                     in0=hTt[:, ND // 2:ND, qsl], in1=rbc_b4)

            def hnorm_gh(qh, p6, selg_s):
                """Z base update ghs = selg + c_gh * h; gpsimd-only, emitted
                after the collectives so it does not delay their triggers."""
                qsl = slice(qh * NQH, qh * NQH + NQH)
                for dt in range(ND):
                    gh_t = p6.tile([128, NQH], F32, name="gh_t")
                    nc.gpsimd.tensor_mul(out=gh_t, in0=hTt[:, dt, qsl],
                                         in1=c_gh[:, qsl])
                    nc.gpsimd.tensor_add(out=selg_s[:, dt, qsl], in0=gh_t,
                                         in1=selg_s[:, dt, qsl])

            def mlp_gateup(qh, p7, fc_lo, fc_hi, hooks=None):
                qsl = slice(qh * NQH, qh * NQH + NQH)
                for fc in range(fc_lo, fc_hi):
                    if hooks is not None and fc in hooks:
                        hooks[fc]()
                    gps = ps.tile([128, 512], F32, name="gps", tag="ps")
                    ups = ps.tile([128, 512], F32, name="ups", tag="ps")
                    for kk in range(ND // 2):
                        nc.tensor.matmul(
                            gps[:, 0:NQH], lhsT=w_g[:, fc, 2 * kk:2 * kk + 2, :],
                            rhs=n2T[:, 2 * kk:2 * kk + 2, qsl],
                            start=(kk == 0), stop=(kk == ND // 2 - 1),
                            perf_mode=PM.DoubleRow)
                    for kk in range(ND // 2):
                        nc.tensor.matmul(
                            ups[:, 0:NQH], lhsT=w_u[:, fc, 2 * kk:2 * kk + 2, :],
                            rhs=n2T[:, 2 * kk:2 * kk + 2, qsl],
                            start=(kk == 0), stop=(kk == ND // 2 - 1),
                            perf_mode=PM.DoubleRow)
                    sg = p7.tile([128, NQH], BF16, name="sg", bufs=4)
                    nc.scalar.activation(out=sg, in_=gps[:, 0:NQH], func=AF.Silu,
                                         scale=1.0 / S_G)
                    nc.vector.tensor_mul(out=actT[:, fc, qsl],
                                         in0=ups[:, 0:NQH], in1=sg)

            def mlp_down(qh, p7, selg_s):
                qsl = slice(qh * NQH, qh * NQH + NQH)
                for dc in range(ND):
                    mps = ps.tile([128, 512], F32, name="mps", tag="ps")
                    for kk in range(NFL // 2):
                        nc.tensor.matmul(
                            mps[:, 0:NQH], lhsT=w_d[:, dc, 2 * kk:2 * kk + 2, :],
                            rhs=actT[:, 2 * kk:2 * kk + 2, qsl],
                            start=(kk == 0), stop=(kk == NFL // 2 - 1),
                            perf_mode=PM.DoubleRow)
                    f1 = p7.tile([128, NQH], BF16, name="f1", bufs=8)
                    nc.vector.tensor_mul(out=f1, in0=mps[:, 0:NQH],
                                         in1=c_g[:, qsl])
                    nc.vector.tensor_add(out=f1, in0=f1,
                                         in1=selg_s[:, dc, qsl])
                    (nc.gpsimd if dc % 2 == 0 else nc.scalar).dma_start(
                        out=updT[:, dc, qsl], in_=f1)

            # interleave kv chunks with the first attention pass
            with tc.tile_pool(name="ph2", bufs=3) as p2, \
                 tc.tile_pool(name="ph3", bufs=3) as p3:
                cps0 = alloc_cps(0)
                live0 = [t_ for t_ in range(NT) if qlo[(0, t_)] < NQH]
                kv_chunk(0, p2)
                qproj(p3)
                kv_chunk(1, p2)
                attn_tloop(0, cps0, tts=[t_ for t_ in live0 if t_ < 4])
                kv_chunk(2, p2)
                attn_tloop(0, cps0, tts=[t_ for t_ in live0 if 4 <= t_ < 8])
                kv_chunk(3, p2)
                attn_tloop(0, cps0, tts=[t_ for t_ in live0 if t_ >= 8])
                attn_evict(0, cps0)

            pN_cm.__exit__(None, None, None)

            # MLP weights + gating state live in the space freed by pN; the
            # 3 big fp8 transfers stream in under attention pass 1.
            pB_cm = tc.tile_pool(name="pB", bufs=1)
            pB = pB_cm.__enter__()
            w_g = pB.tile([128, NFL, ND, 128], FP8, name="w_g")
            w_u = pB.tile([128, NFL, ND, 128], FP8, name="w_u")
            w_d = pB.tile([128, ND, NFL, 128], FP8, name="w_d")
            selg_s = pB.tile([128, ND, NQ], F32, name="selg_s")
            def load_mlp_w():
                # issued from inside attention pass 1: their SBUF region
                # reuses xn's (so the transfers cannot start before the kv
                # phase drains anyway), and the scalar queue is clear of
                # latency-critical DMAs from here to the end of the pass.
                nc.scalar.dma_start(out=selg_s, in_=selg)
                nc.scalar.dma_start(out=w_g, in_=gw)
                nc.scalar.dma_start(out=w_u, in_=uw)
                nc.scalar.dma_start(out=w_d, in_=dw)

            with tc.tile_pool(name="ph5", bufs=1) as p5, \
                 tc.tile_pool(name="ph6", bufs=2) as p6, \
                 tc.tile_pool(name="ph7", bufs=2) as p7:
                cps1 = alloc_cps(1)
                attn_tloop(1, cps1,
                           hooks={0: lambda: oproj(0, p5),
                                  1: load_mlp_w})
                attn_evict(1, cps1)
                oproj(1, p5)
                hnorm(0, p6, selg_s)
                hnorm_gh(0, p6, selg_s)
                mlp_gateup(0, p7, 0, NFL,
                           hooks={10: lambda: hnorm(1, p6, selg_s)})
                mlp_down(0, p7, selg_s)
                hnorm_gh(1, p6, selg_s)
                mlp_gateup(1, p7, 0, NFL)
                mlp_down(1, p7, selg_s)

            pB_cm.__exit__(None, None, None)
            p4_cm.__exit__(None, None, None)
            pA_cm.__exit__(None, None, None)

    _split_excess_waits(nc)
    return nc


# ---------------------------------------------------------------------------
# host side
# ---------------------------------------------------------------------------

def _bf16(x):
    return np.asarray(x, dtype=np.float32).astype(ml_dtypes.bfloat16)


def _fp8(x):
    return np.asarray(x, dtype=np.float32).astype(ml_dtypes.float8_e4m3fn)


def _rope_matrix():
    """R[k, p] = sign(p) * 1[k == swap(p)]; (R.T @ x)[p] = sign(p)*x[swap(p)]."""
    R = np.zeros((128, 128), np.float32)
    for p in range(128):
        base = (p // 64) * 64
        off = p % 64
        if off < 32:
            R[base + off + 32, p] = -1.0
        else:
            R[base + off - 32, p] = 1.0
    return R


def _install_ntff_hook():
    """Shim antenv.axon_hooks (absent in this image) so trace=True works."""
    import types
    try:
        import antenv.axon_hooks  # noqa: F401
        return
    except ImportError:
        pass
    try:
        from trn_agent_boot.trn_boot import _ntff_profile_via_ctypes
        hook = _ntff_profile_via_ctypes("/opt/axon/libaxon_pjrt.so")
    except Exception:
        hook = None
    mod = types.ModuleType("antenv.axon_hooks")
    mod._hook = hook
    mod.set_axon_ntff_profile_hook = lambda h: setattr(mod, "_hook", h)
    mod.get_axon_ntff_profile_hook = lambda: mod._hook
    sys.modules["antenv.axon_hooks"] = mod


def kernel(hidden_states, token_indices, batch_indices, gating_scores, cos, sin,
           ln1_w, ln2_w, q_w, q_b, k_w, k_b, v_w, v_b, o_w, gate_w, up_w, down_w,
           _profile=False, _dbg=False):
    hidden_states = np.asarray(hidden_states, dtype=np.float32)
    token_indices = np.asarray(token_indices).astype(np.int64)
    gating_scores = np.asarray(gating_scores, dtype=np.float32)
    cos = np.asarray(cos, dtype=np.float32)
    sin = np.asarray(sin, dtype=np.float32)
    ln1_w = np.asarray(ln1_w, dtype=np.float32)
    ln2_w = np.asarray(ln2_w, dtype=np.float32)

    topk = token_indices.reshape(B, KSEL)
    gsc = gating_scores.reshape(B, KSEL)

    qlo, qhi = {}, {}
    for qh in range(2):
        for tt in range(NT):
            los, his = [], []
            for b in range(B):
                pos_q = np.asarray(topk[b, qh * NQH:(qh + 1) * NQH])
                los.append(int(np.searchsorted(pos_q, tt * 128)))
                his.append(int(np.searchsorted(pos_q, tt * 128 + 126,
                                               side="right")))
            qlo[(qh, tt)] = min(los)
            qhi[(qh, tt)] = max(his)

    nc = build_program(qlo, qhi, dbg=_dbg)

    q_w_eff = (np.asarray(q_w, np.float32) * ln1_w[None, :]) / 8.0
    k_w_eff = np.asarray(k_w, np.float32) * ln1_w[None, :]
    v_w_eff = np.asarray(v_w, np.float32) * ln1_w[None, :]
    g_w_eff = np.asarray(gate_w, np.float32) * ln2_w[None, :] * S_G
    u_w_eff = np.asarray(up_w, np.float32) * ln2_w[None, :] * S_U
    q_b_eff = np.asarray(q_b, np.float32) / 8.0
    down_f = np.asarray(down_w, np.float32) * S_D

    tvals = (np.arange(NT)[None, :] * 128 + np.arange(128)[:, None]).astype(np.float32)
    rope_m = _rope_matrix()

    def pmaj(a):
        """[c, 128, x] -> [128, c, x] partition-major."""
        return np.ascontiguousarray(a.transpose(1, 0, 2))

    # per-half shards: attention heads AND d_ff halves keyed by rank hh
    half_w = []
    for hh in range(2):
        qsl = slice(hh * HL * HD, (hh + 1) * HL * HD)
        ksl = slice(hh * KVL * HD, (hh + 1) * KVL * HD)
        fsl = slice(hh * (DFF // 2), (hh + 1) * (DFF // 2))
        qwT = _bf16(pmaj(q_w_eff.T[:, qsl][:, HEAD_PERM_L]
                         .reshape(ND, 128, HL * HD)))
        kwT = _bf16(pmaj(k_w_eff.T[:, ksl].reshape(ND, 128, KVL * HD)))
        vwT = _bf16(pmaj(v_w_eff.T[:, ksl].reshape(ND, 128, KVL * HD)))
        owT = _bf16(pmaj(np.asarray(o_w, np.float32).T[qsl, :][HEAD_PERM_L, :]
                         .reshape(NQC, 128, D)))
        qb_a = np.ascontiguousarray(
            q_b_eff[qsl][HEAD_PERM_L].reshape(NQC, 128).T).astype(np.float32)
        kb_a = np.ascontiguousarray(
            np.asarray(k_b, np.float32)[ksl].reshape(NKC, 128).T)
        vb_a = np.broadcast_to(np.asarray(v_b, np.float32)[ksl][None, :],
                               (128, KVL * HD))
        gwa = _fp8(np.ascontiguousarray(
            g_w_eff[fsl].reshape(NFL, 128, ND, 128).transpose(3, 0, 2, 1)))
        uwa = _fp8(np.ascontiguousarray(
            u_w_eff[fsl].reshape(NFL, 128, ND, 128).transpose(3, 0, 2, 1)))
        dwa = _fp8(np.ascontiguousarray(
            down_f[:, fsl].reshape(ND, 128, NFL, 128).transpose(3, 0, 2, 1)))
        half_w.append(dict(qwT=qwT, kwT=kwT, vwT=vwT, owT=owT,
                           gw=gwa, uw=uwa, dw=dwa,
                           _qb=qb_a, _kb=kb_a, _vb=vb_a))

    def stack2(mat):
        mT = mat.T.astype(np.float32)
        return np.concatenate([mT, mT], axis=0)

    def rms_rows(x):
        v = np.mean(x * x, axis=-1, keepdims=True)
        return x / np.sqrt(v + EPS)

    in_maps = []
    zeros_selg = np.zeros((128, ND, NQ), np.float32)
    zeros_gh = np.zeros((128, NQ), np.float32)
    for c in range(NCORES):
        b = c // 2
        hh = c % 2
        pos_all = np.asarray(topk[b], dtype=np.int64)
        g_all = gsc[b]
        sel = hidden_states[b][pos_all]
        xn_host = rms_rows(hidden_states[b]) * ln1_w
        nsel_host = rms_rows(sel) * ln1_w
        hw = half_w[hh]
        im = {k: v for k, v in hw.items() if not k.startswith("_")}
        g_bc = np.broadcast_to(g_all.astype(np.float32)[None, :], (128, NQ))
        posq = np.broadcast_to(pos_all.astype(np.float32)[None, :], (128, NQ))
        cstF = np.concatenate(
            [hw["_qb"], hw["_kb"], hw["_vb"], posq, tvals,
             g_bc / (S_U * S_D),
             g_bc if hh == 0 else zeros_gh], axis=1).astype(np.float32)
        ident64 = np.zeros((128, 64), np.float32)
        ident64[np.arange(64), np.arange(64)] = 1.0
        cstB = _bf16(np.concatenate(
            [rope_m, stack2(cos[b][pos_all]), stack2(sin[b][pos_all]),
             ident64, stack2(cos[b]), stack2(sin[b])], axis=1))
        im.update(
            xnC=_bf16(xn_host.T.reshape(ND, 128, 4, 512).transpose(2, 1, 0, 3)),
            nselT=_bf16(pmaj(nsel_host.T.reshape(ND, 128, NQ))),
            selO=_bf16(pmaj(sel.T.reshape(ND, 128, NQ))),
            cstF=np.ascontiguousarray(cstF),
            cstB=np.ascontiguousarray(cstB),
            selg=pmaj((sel * (1.0 - g_all)[:, None]).T.reshape(ND, 128, NQ)
                      ).astype(np.float32) if hh == 0 else zeros_selg,
        )
        in_maps.append(im)

    if _profile:
        _install_ntff_hook()
    res = run_bass_kernel_spmd(nc, in_maps, core_ids=list(range(NCORES)),
                               trace=_profile)

    out = hidden_states.copy()
    for pr in range(B):
        z0 = np.asarray(res.results[2 * pr]["updT"],
                        np.float32).transpose(1, 0, 2).reshape(D, NQ).T
        z1 = np.asarray(res.results[2 * pr + 1]["updT"],
                        np.float32).transpose(1, 0, 2).reshape(D, NQ).T
        out[pr, np.asarray(topk[pr]), :] = z0 + z1
    if _profile or _dbg:
        return out, res
    return out


# revision 94
# speedup vs baseline: 1.0796x; 1.0796x over previous
"""Trainium2 Bass kernel for nn_DynamicBlock (sparse-token attention + MLP block).

Contract: kernel(**inputs) takes the FULL unsharded inputs (as produced by
reference.setup_inputs()) and returns the FULL [B, T, D] output.

Sharding (pairwise tensor-parallel): 8 cores = 4 batches x 2 halves.
Each core of a batch pair:
 - K/V projections (+rope on K) over all T for its 4 kv-heads, interleaved
   with the first attention pass to hide the hidden-state DMA stream,
 - Q proj + rope for its 8 q-heads over ALL 512 selected queries,
 - causal attention (its heads, all 512 queries) one 256-query half at a
   time; after each half: o-proj partial over its heads' o_w columns and a
   2-rank bf16 AllReduce of that half's partial attn_out (AR of half A
   overlaps the attention of half B; AR of B overlaps the MLP on A),
 - h = AR-sum + residual, rmsnorm2, then MLP over its d_ff HALF (16 of 32
   ff-chunks) for ALL 512 tokens, emitting the partial gated update
   Z_r = selg_r + g*h*alpha_r + g*mlp_r (alpha = 1 on rank 0, 0 on rank 1),
 - host sums Z_0 + Z_1 per pair and scatters into hidden_states.

MLP runs in fp8e4 (DoubleRow perf mode, 2x PE throughput): gate/up/down
weights are pre-scaled powers of two chosen to keep fp8 operands in normal
range, activations (n2, act) are quantized on the fly, and the combined
scale is folded into the host-side gating multiplier. MLP weights are
DMA'd once into SBUF (3 large transfers issued at kernel start, landing
during the attention phase) and reused for both query halves.

Softmax normalization uses the scalar engine's Reciprocal activation read
directly from the PSUM row-sum row (partition 64, a legal PE base) plus a
1xN bf16 broadcast matmul; rmsnorm2 uses Rsqrt the same way. Both avoid
the slow DVE reciprocal and SBUF->SBUF descriptor hops.

Everything on-device runs in a transposed layout ([feature, token]);
rotate_half for rope is a PE matmul with a signed permutation matrix.
"""

import sys

sys.path.insert(0, "/opt/trn_rl_repo")

import numpy as np
import ml_dtypes

import concourse.bass as bass
import concourse.tile as tile
from concourse import mybir
from concourse import bass_utils as _bu
from concourse.bass_utils import run_bass_kernel_spmd
from concourse.vector_clock import ScopedClock, VectorClock



BF16 = mybir.dt.bfloat16
F32 = mybir.dt.float32
FP8 = mybir.dt.float8e4
AF = mybir.ActivationFunctionType
OP = mybir.AluOpType
PM = mybir.MatmulPerfMode

B, T, D = 4, 2048, 1024
H, KV, HD = 16, 8, 64
DFF = 4096
KSEL = 512
EPS = 1e-6

NQ = 512          # selected queries per batch (all of them, head-split)
NQH = 256         # query half processed per attention pass
ND = D // 128     # 8 d-tiles
NT = T // 128     # 16 key tiles
HL = H // 2       # 8 local q heads
KVL = KV // 2     # 4 local kv heads
NKC = KVL * HD // 128  # 2 local k-output chunks (2 kv heads each)
NQC = HL * HD // 128   # 4 local q-output chunks (2 q heads each)
NFC = DFF // 128       # 32 ff chunks
NFL = NFC // 2         # 16 local ff chunks (d_ff tensor-parallel)
NCORES = 8
PAIRS = [[0, 1], [2, 3], [4, 5], [6, 7]]

# fp8 weight scales (powers of two; folded into host-side gate multiplier)
S_G = 512.0
S_U = 16.0
S_D = 512.0

# local q-head layout: q-chunk 2c holds local heads (4c, 4c+2) on partition
# halves (local kv heads 2c / 2c+1), chunk 2c+1 holds (4c+1, 4c+3).
TILE_HEADS_L = []
for c in range(2):
    TILE_HEADS_L.append((4 * c, 4 * c + 2))
    TILE_HEADS_L.append((4 * c + 1, 4 * c + 3))
HEAD_PERM_L = np.array(
    [h * HD + i for pair in TILE_HEADS_L for h in pair for i in range(HD)])


# ---------------------------------------------------------------------------
# walrus workarounds: this toolchain encodes at most ONE semaphore wait per
# instruction. Split the tile tail-drain into per-proc drains and move excess
# waits onto NoOps.
# ---------------------------------------------------------------------------

def _patched_drain_and_barrier(self, tick_clock, wait_clock):
    gc = tick_clock.global_clock
    n = len(gc)
    for i in range(n):
        t = gc[i]
        if t > 0:
            vec = [0] * n
            vec[i] = t
            d = self.nc.sync.drain()
            wait_clock.add_sem_waits(d.ins, ScopedClock({None: VectorClock(vec)}))
    self.nc.all_engine_barrier()
    popped = self.nc._tile_sem_poison_stack.pop()
    assert popped is self._sem_poison
    self.nc.clear_and_free_semaphores(list(self.sems.allocated().values()))


tile.TileContext._drain_and_barrier = _patched_drain_and_barrier

_MAX_WAITS = 1


def _split_excess_waits(nc):
    for f in nc.m.functions:
        for bb in f.blocks:
            new = []
            for inst in bb.instructions:
                si = inst.sync_info
                if si is not None and si.on_wait is not None and len(si.on_wait) > _MAX_WAITS:
                    waits = list(si.on_wait)
                    excess, keep = waits[:-_MAX_WAITS], waits[-_MAX_WAITS:]
                    k = 0
                    while excess:
                        chunk, excess = excess[:_MAX_WAITS], excess[_MAX_WAITS:]
                        new.append(mybir.InstNoOp(
                            name=f"{inst.name}_ws{k}",
                            engine=inst.engine,
                            sync_info=mybir.SyncInfo(on_wait=chunk, on_update=[])))
                        k += 1
                    inst.sync_info = mybir.SyncInfo(
                        on_wait=keep, on_update=list(si.on_update or []))
                new.append(inst)
            bb.instructions = new


def act_unchecked(eng, out, in_, func, bias=0.0, scale=1.0):
    """scalar.activation without the Reciprocal/Rsqrt accuracy guard (our
    tolerance is 2e-2; the LUT error is ~1e-3)."""
    inputs = [eng.lower_ap(in_)]
    for arg in [bias, scale, 0.0]:
        if isinstance(arg, bass.AP):
            inputs.append(eng.lower_ap(arg))
        else:
            inputs.append(mybir.ImmediateValue(dtype=mybir.dt.float32, value=arg))
    outputs = [eng.lower_ap(out)]
    return eng.add_instruction(
        mybir.InstActivation(
            name=eng.bass.get_next_instruction_name(),
            func=func, ins=inputs, outs=outputs))


# ---------------------------------------------------------------------------
# device program
# ---------------------------------------------------------------------------

def build_program(qlo, qhi, dbg=False):
    """qlo/qhi: dict[(qh, tt)] compile-time query ranges within each 256-query
    half (uniform across cores/batches)."""
    nc = bass.Bass(trn_type="TRN2", target_bir_lowering=False, debug=False)

    def inp(name, shape, dt):
        return nc.dram_tensor(name, shape, dt, kind="ExternalInput").ap()

    # ALL inputs are host-pre-arranged partition-major so every DMA is a
    # linear copy (128 descriptors of 4-32KB); strided/transposing DMAs are
    # descriptor-rate bound (~8.5ns/descriptor) and 6-8x slower.
    xnC = inp("xnC", [4, 128, ND, 512], BF16)     # normalized hidden.T, chunked
    nselT = inp("nselT", [128, ND, NQ], BF16)     # host-normalized selected.T
    selO = inp("selO", [128, ND, NQ], BF16)       # raw selected rows.T
    qwT = inp("qwT", [128, ND, HL * HD], BF16)
    kwT = inp("kwT", [128, ND, KVL * HD], BF16)
    vwT = inp("vwT", [128, ND, KVL * HD], BF16)
    owT = inp("owT", [128, NQC, D], BF16)
    gw = inp("gw", [128, NFL, ND, 128], FP8)
    uw = inp("uw", [128, NFL, ND, 128], FP8)
    dw = inp("dw", [128, ND, NFL, 128], FP8)
    # packed small constants: f32 block and bf16 block (one DMA each)
    NCF = 4 + 2 + KVL * HD + NQ + NT + NQ + NQ
    NCB = 128 + NQ + NQ + 64 + 2 * T
    cstF = inp("cstF", [128, NCF], F32)
    cstB = inp("cstB", [128, NCB], BF16)
    selg = inp("selg", [128, ND, NQ], F32)  # selres*(1-g) on rank 0, zeros rank 1

    updT = nc.dram_tensor("updT", [128, ND, NQ], BF16,
                          kind="ExternalOutput").ap()

    with tile.TileContext(nc, pool_alloc_mode="queue") as tc:
        with tc.tile_pool(name="ps", bufs=8, space="PSUM") as ps, \
             tc.tile_pool(name="persist", bufs=1) as pp, \
             tc.tile_pool(name="rows", bufs=2) as rowp, \
             tc.tile_pool(name="dramp", bufs=1, space="DRAM") as dram:

            # AllGather of fp8 partials + local add beats AllReduce: the CC
            # cost is a fixed ~15us overhead plus bytes moved at ~30GB/s, and
            # AR pays a 1.875x protocol multiplier on top.
            cc_in = [dram.tile([128, ND, NQH], FP8, name=f"cc_in{i}")
                     for i in range(2)]
            cc_out = [dram.tile([2, 128, ND, NQH], FP8, name=f"cc_out{i}")
                      for i in range(2)]

            # ---- persistent tiles ------------------------------------------
            hTt = pp.tile([128, ND, NQ], BF16, name="hTt")
            n2T = pp.tile([128, ND, NQ], FP8, name="n2T")
            ctxT = pp.tile([128, NQC, NQ], BF16, name="ctxT")
            actT = pp.tile([128, NFL, NQ], FP8, name="actT")
            ones_t = pp.tile([128, 1], BF16, name="ones_t")
            nc.vector.memset(ones_t, 1.0)
            eps_t = pp.tile([1, 1], F32, name="eps_t")
            nc.vector.memset(eps_t, EPS)
            ones65 = pp.tile([65, 128], BF16, name="ones65")
            nc.vector.memset(ones65, 1.0)

            cF = pp.tile([128, NCF], F32, name="cF")
            cB = pp.tile([128, NCB], BF16, name="cB")
            o_ = 0
            c_qb = cF[:, o_:o_ + NQC]; o_ += NQC
            c_kb = cF[:, o_:o_ + NKC]; o_ += NKC
            c_vb = cF[:, o_:o_ + KVL * HD]; o_ += KVL * HD
            c_pos = cF[:, o_:o_ + NQ]; o_ += NQ
            c_tv = cF[:, o_:o_ + NT]; o_ += NT
            c_g = cF[:, o_:o_ + NQ]; o_ += NQ
            c_gh = cF[:, o_:o_ + NQ]; o_ += NQ
            assert o_ == NCF
            c_rm = cB[:, 0:128]
            c_cq = cB[:, 128:128 + NQ]
            c_sq = cB[:, 128 + NQ:128 + 2 * NQ]
            ident64 = cB[:, 128 + 2 * NQ:128 + 2 * NQ + 64]  # I on parts 0:64
            _o2 = 128 + 2 * NQ + 64
            c_ck = cB[:, _o2:_o2 + T]
            c_sk = cB[:, _o2 + T:_o2 + 2 * T]
            pA_cm = tc.tile_pool(name="pA", bufs=1)
            pA = pA_cm.__enter__()
            kT = pA.tile([128, NKC, T], BF16, name="kT")
            vplus = pA.tile([128, NT, KVL, HD + 1], BF16, name="vplus")
            nc.vector.memset(vplus[:, :, :, HD:HD + 1], 1.0)
            qrT = pA.tile([128, NQC, NQ], BF16, name="qrT")
            w_o = pA.tile([128, NQC, D], BF16, name="w_o")
            selOs = pA.tile([128, ND, NQ], BF16, name="selOs")

            p4_cm = tc.tile_pool(name="ph4", bufs=1)
            p4 = p4_cm.__enter__()

            pN_cm = tc.tile_pool(name="pN", bufs=1)
            pN = pN_cm.__enter__()
            xn = pN.tile([128, 4, ND, 512], BF16, name="xn")
            w_k = pN.tile([128, ND, KVL * HD], BF16, name="w_k")
            w_v = pN.tile([128, ND, KVL * HD], BF16, name="w_v")
            w_q = pN.tile([128, ND, HL * HD], BF16, name="w_q")
            nsel = pN.tile([128, ND, NQ], BF16, name="nsel")

            # ---- input DMAs (all linear; issue order = priority per engine;
            # xn chunks spread over the three DMA queues) ---
            nc.gpsimd.dma_start(out=w_k[:, :, 0:128], in_=kwT[:, :, 0:128])
            nc.gpsimd.dma_start(out=w_k[:, :, 128:256], in_=kwT[:, :, 128:256])
            nc.gpsimd.dma_start(out=w_v, in_=vwT)
            nc.sync.dma_start(out=xn[:, 0, 0:ND // 2], in_=xnC[0][:, 0:ND // 2])
            nc.sync.dma_start(out=xn[:, 0, ND // 2:ND],
                              in_=xnC[0][:, ND // 2:ND])
            nc.sync.dma_start(out=xn[:, 3], in_=xnC[3])
            nc.scalar.dma_start(out=cF, in_=cstF)
            nc.scalar.dma_start(out=cB, in_=cstB)
            nc.scalar.dma_start(out=xn[:, 1], in_=xnC[1])
            nc.gpsimd.dma_start(out=nsel, in_=nselT)
            nc.gpsimd.dma_start(out=w_q, in_=qwT)
            nc.gpsimd.dma_start(out=xn[:, 2], in_=xnC[2])
            nc.gpsimd.dma_start(out=w_o, in_=owT)
            nc.gpsimd.dma_start(out=selOs, in_=selO)

            # ==================================================================
            # Phase 1: K (+rope) and V per 512-token chunk, interleaved with
            # the first attention t-loop to hide the xn DMA stream.
            # ==================================================================
            def kv_chunk(ch, p2):
                    cs = slice(ch * 512, (ch + 1) * 512)
                    for kc in range(NKC):
                        kps = ps.tile([128, 512], F32, name="kps", tag="ps")
                        for dt in range(ND):
                            nc.tensor.matmul(
                                kps, lhsT=w_k[:, dt, kc * 128:(kc + 1) * 128],
                                rhs=xn[:, ch, dt, :],
                                start=(dt == 0), stop=(dt == ND - 1))
                        kraw = p2.tile([128, 512], BF16, name="kraw")
                        nc.vector.tensor_scalar(
                            out=kraw, in0=kps, scalar1=c_kb[:, kc:kc + 1],
                            scalar2=None, op0=OP.add)
                        rot = ps.tile([128, 512], F32, name="rot", tag="ps")
                        nc.tensor.matmul(rot, lhsT=c_rm, rhs=kraw,
                                         start=True, stop=True)
                        dst = kT[:, kc, cs]
                        tmp = p2.tile([128, 512], BF16, name="tmp")
                        nc.vector.tensor_mul(out=tmp, in0=rot, in1=c_sk[:, cs])
                        nc.vector.tensor_mul(out=dst, in0=kraw, in1=c_ck[:, cs])
                        nc.vector.tensor_add(out=dst, in0=dst, in1=tmp)

                    for tt in range(ch * 4, ch * 4 + 4):
                        vps = ps.tile([128, 512], F32, name="vps", tag="ps")
                        to = (tt % 4) * 128
                        for dt in range(ND):
                            nc.tensor.matmul(
                                vps[:, 0:KVL * HD],
                                lhsT=xn[:, ch, dt, to:to + 128],
                                rhs=w_v[:, dt, :],
                                start=(dt == 0), stop=(dt == ND - 1))
                        nc.vector.tensor_add(
                            out=vplus[:, tt, :, 0:HD],
                            in0=vps[:, 0:KVL * HD].rearrange(
                                "p (h d) -> p h d", h=KVL),
                            in1=c_vb.rearrange("p (h d) -> p h d", h=KVL))

            # ==================================================================
            # Phase 2: Q proj + rope (host-normalized input)
            # ==================================================================
            def qproj(p3):
                for qc in range(NQC):
                    qps = ps.tile([128, 512], F32, name="qps", tag="ps")
                    for dt in range(ND):
                        nc.tensor.matmul(
                            qps[:, 0:NQ], lhsT=w_q[:, dt, qc * 128:(qc + 1) * 128],
                            rhs=nsel[:, dt, :],
                            start=(dt == 0), stop=(dt == ND - 1))
                    qraw = p3.tile([128, NQ], BF16, name="qraw")
                    nc.vector.tensor_scalar(
                        out=qraw, in0=qps[:, 0:NQ], scalar1=c_qb[:, qc:qc + 1],
                        scalar2=None, op0=OP.add)
                    rotq = ps.tile([128, 512], F32, name="rotq", tag="ps")
                    nc.tensor.matmul(rotq[:, 0:NQ], lhsT=c_rm, rhs=qraw,
                                     start=True, stop=True)
                    dst = qrT[:, qc, :]
                    tmpq = p3.tile([128, NQ], BF16, name="tmpq")
                    nc.vector.tensor_mul(out=tmpq, in0=rotq[:, 0:NQ], in1=c_sq)
                    nc.vector.tensor_mul(out=dst, in0=qraw, in1=c_cq)
                    nc.vector.tensor_add(out=dst, in0=dst, in1=tmpq)

            # ==================================================================
            # Phases 3-7 per query half: attention t-loop, eviction, o-proj +
            # AllReduce (overlapped), h + rmsnorm2, d_ff-split MLP + Z output.
            # ==================================================================

            def alloc_cps(qh):
                # one accumulator per (kc, kv-half); free dim packs BOTH
                # q-head (ab) blocks so the ctx matmul is a single 3D-out
                # instruction per (tt, kc, half)
                cps = {}
                for kc in range(NKC):
                    for hf in range(2):
                        cps[(kc, hf)] = ps.tile([128, 512], F32,
                                                name=f"cps{qh}{kc}{hf}",
                                                tag="ps")
                return cps

            def attn_tloop(qh, cps, tts=None, hooks=None):
                qs0 = qh * NQH
                live = [t_ for t_ in range(NT) if qlo[(qh, t_)] < NQH]
                last_tt = max(live)
                for ti, tt in enumerate(live if tts is None else tts):
                    if hooks is not None and ti in hooks:
                        hooks[ti]()
                    lo = qlo[(qh, tt)]
                    hi = qhi[(qh, tt)]
                    mask = None
                    if hi > lo:
                        mask = p4.tile([128, 512], BF16, name="mask", bufs=2)
                        for mh in range(2):
                            nc.vector.tensor_scalar(
                                out=mask[:, mh * NQH + lo:mh * NQH + hi],
                                in0=c_pos[:, qs0 + lo:qs0 + hi],
                                scalar1=c_tv[:, tt:tt + 1], scalar2=None,
                                op0=OP.is_ge)
                    # stage-major within the t-tile: all scores first, then
                    # all ctx matmuls — the exps pipeline on the scalar
                    # engine behind the remaining scores instead of
                    # serializing a score->exp->ctx ring per head group.
                    ptl = []
                    for kc in range(NKC):
                        for half in range(2):
                            hs_ = slice(half * 64, (half + 1) * 64)
                            sp = ps.tile([128, 512], F32, name="sp", tag="ps")
                            # one matmul streams both q-head chunks (3D rhs)
                            nc.tensor.matmul(
                                sp.rearrange("p (a q) -> p a q", a=2)[:, :, lo:NQH],
                                lhsT=kT[hs_, kc, tt * 128:(tt + 1) * 128],
                                rhs=qrT[hs_, 2 * kc:2 * kc + 2,
                                        qs0 + lo:qs0 + NQH],
                                start=True, stop=True)
                            pt = p4.tile([128, 2, NQH], BF16, name="pt", bufs=8)
                            nc.scalar.activation(
                                out=pt[:, :, lo:NQH],
                                in_=sp.rearrange("p (h q) -> p h q", h=2)[:, :, lo:NQH],
                                func=AF.Exp)
                            if mask is not None:
                                nc.vector.tensor_mul(
                                    out=pt[:, :, lo:hi],
                                    in0=pt[:, :, lo:hi],
                                    in1=mask.rearrange(
                                        "p (h q) -> p h q", h=2)[:, :, lo:hi])
                            ptl.append((kc, half, pt))
                    for kc, half, pt in ptl:
                        cp = cps[(kc, half)]
                        nc.tensor.matmul(
                            cp.rearrange("p (a q) -> p a q",
                                         a=2)[0:HD + 1, :, lo:NQH],
                            lhsT=vplus[:, tt, 2 * kc + half, 0:HD + 1],
                            rhs=pt[:, :, lo:NQH],
                            start=(tt == 0), stop=(tt == last_tt))

            def attn_evict(qh, cps):
                """ctx rows sit at PSUM partitions 0..63, the row-sum at
                partition 64 (a legal PE base). Reciprocal on the scalar
                engine straight from PSUM, 1x64 bf16 broadcast matmul, then
                per-head scaling. Second head of each pair staged and moved
                with a single SBUF->SBUF DMA."""
                qsl = slice(qh * NQH, qh * NQH + NQH)
                tiles = [(k_, h_) for k_ in range(NKC) for h_ in range(2)]
                # stage-major emission: all recips, then all broadcasts, ...
                # so the four tiles pipeline across engines instead of
                # serializing one ~1.7us chain per tile.
                rrs, rbs, casts, stages, psbs = [], [], {}, [], []
                for kc, hf in tiles:
                    cp = cps[(kc, hf)]
                    rr = p4.tile([65, 512], BF16, name="rr", bufs=4)
                    act_unchecked(nc.scalar, rr[64:65, :], cp[64:65, :],
                                  AF.Reciprocal)
                    rrs.append(rr)
                for g, (kc, hf) in enumerate(tiles):
                    rb = ps.tile([128, 512], F32, name="rb", tag="ps")
                    nc.tensor.matmul(rb[0:64, :], lhsT=ones65[64:65, 0:64],
                                     rhs=rrs[g][64:65, :],
                                     start=True, stop=True)
                    rbs.append(rb)
                for g, (kc, hf) in enumerate(tiles):
                    rb_sb = p4.tile([64, 512], BF16, name="rb_sb", bufs=4)
                    nc.vector.tensor_copy(out=rb_sb, in_=rbs[g][0:64, :])
                    casts[(kc, hf)] = rb_sb
                # output chunk 2kc+ab: head[0] from kv-half-0 tile's ab block,
                # head[1] from kv-half-1 tile's ab block
                chunks = [(k_, a_) for k_ in range(NKC) for a_ in range(2)]
                for kc, ab in chunks:
                    absl = slice(ab * NQH, (ab + 1) * NQH)
                    # second head of the pair staged for the PE identity move
                    # (SBUF->SBUF DMAs are descriptor-rate bound: ~17us)
                    stage = p4.tile([64, NQH], BF16, name="stage", bufs=4)
                    nc.vector.tensor_mul(
                        out=stage,
                        in0=cps[(kc, 1)][0:HD, absl],
                        in1=casts[(kc, 1)][:, absl])
                    stages.append(stage)
                    nc.vector.tensor_mul(
                        out=ctxT[0:64, 2 * kc + ab, qsl],
                        in0=cps[(kc, 0)][0:HD, absl],
                        in1=casts[(kc, 0)][:, absl])
                for g, (kc, ab) in enumerate(chunks):
                    psb = ps.tile([128, 512], F32, name="psb", tag="ps")
                    nc.tensor.matmul(psb[64:128, 0:NQH],
                                     lhsT=ident64[0:64, :], rhs=stages[g],
                                     start=True, stop=True)
                    psbs.append(psb)
                for g, (kc, ab) in enumerate(chunks):
                    nc.vector.tensor_copy(
                        out=ctxT[64:128, 2 * kc + ab, qsl],
                        in_=psbs[g][64:128, 0:NQH])

            def oproj(qh, p5):
                qsl = slice(qh * NQH, qh * NQH + NQH)
                o_st = p5.tile([128, ND, NQH], FP8, name="o_st")
                for dc in range(ND):
                    ops_ = ps.tile([128, 512], F32, name="ops_", tag="ps")
                    for hc in range(NQC):
                        nc.tensor.matmul(
                            ops_[:, 0:NQH],
                            lhsT=w_o[:, hc, dc * 128:(dc + 1) * 128],
                            rhs=ctxT[:, hc, qsl],
                            start=(hc == 0), stop=(hc == NQC - 1))
                    nc.vector.tensor_copy(out=o_st[:, dc, :], in_=ops_[:, 0:NQH])
                nc.sync.dma_start(out=cc_in[qh], in_=o_st)
                nc.gpsimd.collective_compute(
                    "AllGather", OP.bypass, replica_groups=PAIRS,
                    ins=[cc_in[qh].opt()], outs=[cc_out[qh].opt()])

            def hnorm(qh, p6, selg_s):
                """h = AR + residual for this half; rmsnorm2 -> n2T half (fp8);
                Z base ghs = selg + c_gh * h."""
                qsl = slice(qh * NQH, qh * NQH + NQH)
                hsb = p6.tile([128, 2, ND, NQH], FP8, name="hsb")
                nc.sync.dma_start(out=hsb[:, 0], in_=cc_out[qh][0])
                nc.sync.dma_start(out=hsb[:, 1], in_=cc_out[qh][1])
                ssn = ps.tile([128, 512], F32, name="ssn", tag="ps")
                for dt in range(ND):
                    eng = nc.vector if dt % 2 == 0 else nc.gpsimd
                    eng.tensor_add(out=hTt[:, dt, qsl],
                                   in0=hsb[:, 0, dt, :],
                                   in1=hsb[:, 1, dt, :])
                    eng.tensor_add(out=hTt[:, dt, qsl],
                                   in0=hTt[:, dt, qsl],
                                   in1=selOs[:, dt, qsl])
                    sq6 = p6.tile([128, NQH], BF16, name="sq6", bufs=4)
                    eng.tensor_mul(out=sq6, in0=hTt[:, dt, qsl],
                                   in1=hTt[:, dt, qsl])
                    nc.tensor.matmul(ssn[0:1, 0:NQH], lhsT=ones_t, rhs=sq6,
                                     start=(dt == 0), stop=(dt == ND - 1))
                rrow = rowp.tile([1, NQH], BF16, name="rrow", tag="row")
                act_unchecked(nc.scalar, rrow, ssn[0:1, 0:NQH], AF.Rsqrt,
                              bias=eps_t[0:1, 0:1], scale=1.0 / D)
                rbc = ps.tile([128, 512], F32, name="rbc", tag="ps")
                nc.tensor.matmul(rbc[:, 0:NQH], lhsT=ones65[0:1, :], rhs=rrow,
                                 start=True, stop=True)
                rbc_sb = p6.tile([128, NQH], BF16, name="rbc_sb")
                nc.vector.tensor_copy(out=rbc_sb, in_=rbc[:, 0:NQH])
                rbc_b4 = bass.AP(tensor=rbc_sb.tensor, offset=rbc_sb.offset,
                                 ap=[rbc_sb.ap[0], [0, ND // 2], rbc_sb.ap[1]])
                nc.vector.tensor_mul(out=n2T[:, 0:ND // 2, qsl],
                                     in0=hTt[:, 0:ND // 2, qsl], in1=rbc_b4)
                nc.gpsimd.tensor_mul(out=n2T[:, ND // 2:ND, qsl],
                                     in0=hTt[:, ND // 2:ND, qsl], in1=rbc_b4)

            def hnorm_gh(qh, p6, selg_s):
                """Z base update ghs = selg + c_gh * h; gpsimd-only, emitted
                after the collectives so it does not delay their triggers."""
                qsl = slice(qh * NQH, qh * NQH + NQH)
                for dt in range(ND):
                    gh_t = p6.tile([128, NQH], F32, name="gh_t")
                    nc.gpsimd.tensor_mul(out=gh_t, in0=hTt[:, dt, qsl],
                                         in1=c_gh[:, qsl])
                    nc.gpsimd.tensor_add(out=selg_s[:, dt, qsl], in0=gh_t,
                                         in1=selg_s[:, dt, qsl])

            def mlp_gateup(qh, p7, fc_lo, fc_hi, hooks=None):
                qsl = slice(qh * NQH, qh * NQH + NQH)
                for fc in range(fc_lo, fc_hi):
                    if hooks is not None and fc in hooks:
                        hooks[fc]()
                    gps = ps.tile([128, 512], F32, name="gps", tag="ps")
                    ups = ps.tile([128, 512], F32, name="ups", tag="ps")
                    for kk in range(ND // 2):
                        nc.tensor.matmul(
                            gps[:, 0:NQH], lhsT=w_g[:, fc, 2 * kk:2 * kk + 2, :],
                            rhs=n2T[:, 2 * kk:2 * kk + 2, qsl],
                            start=(kk == 0), stop=(kk == ND // 2 - 1),
                            perf_mode=PM.DoubleRow)
                    for kk in range(ND // 2):
                        nc.tensor.matmul(
                            ups[:, 0:NQH], lhsT=w_u[:, fc, 2 * kk:2 * kk + 2, :],
                            rhs=n2T[:, 2 * kk:2 * kk + 2, qsl],
                            start=(kk == 0), stop=(kk == ND // 2 - 1),
                            perf_mode=PM.DoubleRow)
                    sg = p7.tile([128, NQH], BF16, name="sg", bufs=4)
                    nc.scalar.activation(out=sg, in_=gps[:, 0:NQH], func=AF.Silu,
                                         scale=1.0 / S_G)
                    nc.vector.tensor_mul(out=actT[:, fc, qsl],
                                         in0=ups[:, 0:NQH], in1=sg)

            def mlp_down(qh, p7, selg_s):
                qsl = slice(qh * NQH, qh * NQH + NQH)
                for dc in range(ND):
                    mps = ps.tile([128, 512], F32, name="mps", tag="ps")
                    for kk in range(NFL // 2):
                        nc.tensor.matmul(
                            mps[:, 0:NQH], lhsT=w_d[:, dc, 2 * kk:2 * kk + 2, :],
                            rhs=actT[:, 2 * kk:2 * kk + 2, qsl],
                            start=(kk == 0), stop=(kk == NFL // 2 - 1),
                            perf_mode=PM.DoubleRow)
                    f1 = p7.tile([128, NQH], BF16, name="f1", bufs=8)
                    nc.vector.tensor_mul(out=f1, in0=mps[:, 0:NQH],
                                         in1=c_g[:, qsl])
                    nc.vector.tensor_add(out=f1, in0=f1,
                                         in1=selg_s[:, dc, qsl])
                    (nc.gpsimd if dc % 2 == 0 else nc.scalar).dma_start(
                        out=updT[:, dc, qsl], in_=f1)

            # interleave kv chunks with the first attention pass
            with tc.tile_pool(name="ph2", bufs=3) as p2, \
                 tc.tile_pool(name="ph3", bufs=3) as p3:
                cps0 = alloc_cps(0)
                live0 = [t_ for t_ in range(NT) if qlo[(0, t_)] < NQH]
                kv_chunk(0, p2)
                qproj(p3)
                kv_chunk(1, p2)
                attn_tloop(0, cps0, tts=[t_ for t_ in live0 if t_ < 4])
                kv_chunk(2, p2)
                attn_tloop(0, cps0, tts=[t_ for t_ in live0 if 4 <= t_ < 8])
                kv_chunk(3, p2)
                attn_tloop(0, cps0, tts=[t_ for t_ in live0 if t_ >= 8])
                attn_evict(0, cps0)

            pN_cm.__exit__(None, None, None)

            # MLP weights + gating state live in the space freed by pN; the
            # 3 big fp8 transfers stream in under attention pass 1.
            pB_cm = tc.tile_pool(name="pB", bufs=1)
            pB = pB_cm.__enter__()
            w_g = pB.tile([128, NFL, ND, 128], FP8, name="w_g")
            w_u = pB.tile([128, NFL, ND, 128], FP8, name="w_u")
            w_d = pB.tile([128, ND, NFL, 128], FP8, name="w_d")
            selg_s = pB.tile([128, ND, NQ], F32, name="selg_s")
            def load_mlp_w():
                # issued from inside attention pass 1: their SBUF region
                # reuses xn's (so the transfers cannot start before the kv
                # phase drains anyway), and the scalar queue is clear of
                # latency-critical DMAs from here to the end of the pass.
                nc.scalar.dma_start(out=selg_s, in_=selg)
                nc.scalar.dma_start(out=w_g, in_=gw)
                nc.scalar.dma_start(out=w_u, in_=uw)
                nc.scalar.dma_start(out=w_d, in_=dw)

            with tc.tile_pool(name="ph5", bufs=1) as p5, \
                 tc.tile_pool(name="ph6", bufs=2) as p6, \
                 tc.tile_pool(name="ph7", bufs=2) as p7:
                cps1 = alloc_cps(1)
                attn_tloop(1, cps1,
                           hooks={0: lambda: oproj(0, p5),
                                  1: load_mlp_w})
                attn_evict(1, cps1)
                oproj(1, p5)
                hnorm(0, p6, selg_s)
                hnorm_gh(0, p6, selg_s)
                mlp_gateup(0, p7, 0, NFL,
                           hooks={10: lambda: hnorm(1, p6, selg_s)})
                mlp_down(0, p7, selg_s)
                hnorm_gh(1, p6, selg_s)
                mlp_gateup(1, p7, 0, NFL)
                mlp_down(1, p7, selg_s)

            pB_cm.__exit__(None, None, None)
            p4_cm.__exit__(None, None, None)
            pA_cm.__exit__(None, None, None)

    _split_excess_waits(nc)
    return nc


# ---------------------------------------------------------------------------
# host side
# ---------------------------------------------------------------------------

def _bf16(x):
    return np.asarray(x, dtype=np.float32).astype(ml_dtypes.bfloat16)


def _fp8(x):
    return np.asarray(x, dtype=np.float32).astype(ml_dtypes.float8_e4m3fn)


def _rope_matrix():
    """R[k, p] = sign(p) * 1[k == swap(p)]; (R.T @ x)[p] = sign(p)*x[swap(p)]."""
    R = np.zeros((128, 128), np.float32)
    for p in range(128):
        base = (p // 64) * 64
        off = p % 64
        if off < 32:
            R[base + off + 32, p] = -1.0
        else:
            R[base + off - 32, p] = 1.0
    return R


def _install_ntff_hook():
    """Shim antenv.axon_hooks (absent in this image) so trace=True works."""
    import types
    try:
        import antenv.axon_hooks  # noqa: F401
        return
    except ImportError:
        pass
    try:
        from trn_agent_boot.trn_boot import _ntff_profile_via_ctypes
        hook = _ntff_profile_via_ctypes("/opt/axon/libaxon_pjrt.so")
    except Exception:
        hook = None
    mod = types.ModuleType("antenv.axon_hooks")
    mod._hook = hook
    mod.set_axon_ntff_profile_hook = lambda h: setattr(mod, "_hook", h)
    mod.get_axon_ntff_profile_hook = lambda: mod._hook
    sys.modules["antenv.axon_hooks"] = mod


def kernel(hidden_states, token_indices, batch_indices, gating_scores, cos, sin,
           ln1_w, ln2_w, q_w, q_b, k_w, k_b, v_w, v_b, o_w, gate_w, up_w, down_w,
           _profile=False, _dbg=False):
    hidden_states = np.asarray(hidden_states, dtype=np.float32)
    token_indices = np.asarray(token_indices).astype(np.int64)
    gating_scores = np.asarray(gating_scores, dtype=np.float32)
    cos = np.asarray(cos, dtype=np.float32)
    sin = np.asarray(sin, dtype=np.float32)
    ln1_w = np.asarray(ln1_w, dtype=np.float32)
    ln2_w = np.asarray(ln2_w, dtype=np.float32)

    topk = token_indices.reshape(B, KSEL)
    gsc = gating_scores.reshape(B, KSEL)

    qlo, qhi = {}, {}
    for qh in range(2):
        for tt in range(NT):
            los, his = [], []
            for b in range(B):
                pos_q = np.asarray(topk[b, qh * NQH:(qh + 1) * NQH])
                los.append(int(np.searchsorted(pos_q, tt * 128)))
                his.append(int(np.searchsorted(pos_q, tt * 128 + 126,
                                               side="right")))
            qlo[(qh, tt)] = min(los)
            qhi[(qh, tt)] = max(his)

    nc = build_program(qlo, qhi, dbg=_dbg)

    q_w_eff = (np.asarray(q_w, np.float32) * ln1_w[None, :]) / 8.0
    k_w_eff = np.asarray(k_w, np.float32) * ln1_w[None, :]
    v_w_eff = np.asarray(v_w, np.float32) * ln1_w[None, :]
    g_w_eff = np.asarray(gate_w, np.float32) * ln2_w[None, :] * S_G
    u_w_eff = np.asarray(up_w, np.float32) * ln2_w[None, :] * S_U
    q_b_eff = np.asarray(q_b, np.float32) / 8.0
    down_f = np.asarray(down_w, np.float32) * S_D

    tvals = (np.arange(NT)[None, :] * 128 + np.arange(128)[:, None]).astype(np.float32)
    rope_m = _rope_matrix()

    def pmaj(a):
        """[c, 128, x] -> [128, c, x] partition-major."""
        return np.ascontiguousarray(a.transpose(1, 0, 2))

    # per-half shards: attention heads AND d_ff halves keyed by rank hh
    half_w = []
    for hh in range(2):
        qsl = slice(hh * HL * HD, (hh + 1) * HL * HD)
        ksl = slice(hh * KVL * HD, (hh + 1) * KVL * HD)
        fsl = slice(hh * (DFF // 2), (hh + 1) * (DFF // 2))
        qwT = _bf16(pmaj(q_w_eff.T[:, qsl][:, HEAD_PERM_L]
                         .reshape(ND, 128, HL * HD)))
        kwT = _bf16(pmaj(k_w_eff.T[:, ksl].reshape(ND, 128, KVL * HD)))
        vwT = _bf16(pmaj(v_w_eff.T[:, ksl].reshape(ND, 128, KVL * HD)))
        owT = _bf16(pmaj(np.asarray(o_w, np.float32).T[qsl, :][HEAD_PERM_L, :]
                         .reshape(NQC, 128, D)))
        qb_a = np.ascontiguousarray(
            q_b_eff[qsl][HEAD_PERM_L].reshape(NQC, 128).T).astype(np.float32)
        kb_a = np.ascontiguousarray(
            np.asarray(k_b, np.float32)[ksl].reshape(NKC, 128).T)
        vb_a = np.broadcast_to(np.asarray(v_b, np.float32)[ksl][None, :],
                               (128, KVL * HD))
        gwa = _fp8(np.ascontiguousarray(
            g_w_eff[fsl].reshape(NFL, 128, ND, 128).transpose(3, 0, 2, 1)))
        uwa = _fp8(np.ascontiguousarray(
            u_w_eff[fsl].reshape(NFL, 128, ND, 128).transpose(3, 0, 2, 1)))
        dwa = _fp8(np.ascontiguousarray(
            down_f[:, fsl].reshape(ND, 128, NFL, 128).transpose(3, 0, 2, 1)))
        half_w.append(dict(qwT=qwT, kwT=kwT, vwT=vwT, owT=owT,
                           gw=gwa, uw=uwa, dw=dwa,
                           _qb=qb_a, _kb=kb_a, _vb=vb_a))

    def stack2(mat):
        mT = mat.T.astype(np.float32)
        return np.concatenate([mT, mT], axis=0)

    def rms_rows(x):
        v = np.mean(x * x, axis=-1, keepdims=True)
        return x / np.sqrt(v + EPS)

    in_maps = []
    zeros_selg = np.zeros((128, ND, NQ), np.float32)
    zeros_gh = np.zeros((128, NQ), np.float32)
    for c in range(NCORES):
        b = c // 2
        hh = c % 2
        pos_all = np.asarray(topk[b], dtype=np.int64)
        g_all = gsc[b]
        sel = hidden_states[b][pos_all]
        xn_host = rms_rows(hidden_states[b]) * ln1_w
        nsel_host = rms_rows(sel) * ln1_w
        hw = half_w[hh]
        im = {k: v for k, v in hw.items() if not k.startswith("_")}
        g_bc = np.broadcast_to(g_all.astype(np.float32)[None, :], (128, NQ))
        posq = np.broadcast_to(pos_all.astype(np.float32)[None, :], (128, NQ))
        cstF = np.concatenate(
            [hw["_qb"], hw["_kb"], hw["_vb"], posq, tvals,
             g_bc / (S_U * S_D),
             g_bc if hh == 0 else zeros_gh], axis=1).astype(np.float32)
        ident64 = np.zeros((128, 64), np.float32)
        ident64[np.arange(64), np.arange(64)] = 1.0
        cstB = _bf16(np.concatenate(
            [rope_m, stack2(cos[b][pos_all]), stack2(sin[b][pos_all]),
             ident64, stack2(cos[b]), stack2(sin[b])], axis=1))
        im.update(
            xnC=_bf16(xn_host.T.reshape(ND, 128, 4, 512).transpose(2, 1, 0, 3)),
            nselT=_bf16(pmaj(nsel_host.T.reshape(ND, 128, NQ))),
            selO=_bf16(pmaj(sel.T.reshape(ND, 128, NQ))),
            cstF=np.ascontiguousarray(cstF),
            cstB=np.ascontiguousarray(cstB),
            selg=pmaj((sel * (1.0 - g_all)[:, None]).T.reshape(ND, 128, NQ)
                      ).astype(np.float32) if hh == 0 else zeros_selg,
        )
        in_maps.append(im)

    if _profile:
        _install_ntff_hook()
    res = run_bass_kernel_spmd(nc, in_maps, core_ids=list(range(NCORES)),
                               trace=_profile)

    out = hidden_states.copy()
    for pr in range(B):
        z0 = np.asarray(res.results[2 * pr]["updT"],
                        np.float32).transpose(1, 0, 2).reshape(D, NQ).T
        z1 = np.asarray(res.results[2 * pr + 1]["updT"],
                        np.float32).transpose(1, 0, 2).reshape(D, NQ).T
        out[pr, np.asarray(topk[pr]), :] = z0 + z1
    if _profile or _dbg:
        return out, res
    return out


# revision 95
# speedup vs baseline: 1.0969x; 1.0161x over previous
"""Trainium2 Bass kernel for nn_DynamicBlock (sparse-token attention + MLP block).

Contract: kernel(**inputs) takes the FULL unsharded inputs (as produced by
reference.setup_inputs()) and returns the FULL [B, T, D] output.

Sharding (pairwise tensor-parallel): 8 cores = 4 batches x 2 halves.
Each core of a batch pair:
 - K/V projections (+rope on K) over all T for its 4 kv-heads, interleaved
   with the first attention pass to hide the hidden-state DMA stream,
 - Q proj + rope for its 8 q-heads over ALL 512 selected queries,
 - causal attention (its heads, all 512 queries) one 256-query half at a
   time; after each half: o-proj partial over its heads' o_w columns and a
   2-rank bf16 AllReduce of that half's partial attn_out (AR of half A
   overlaps the attention of half B; AR of B overlaps the MLP on A),
 - h = AR-sum + residual, rmsnorm2, then MLP over its d_ff HALF (16 of 32
   ff-chunks) for ALL 512 tokens, emitting the partial gated update
   Z_r = selg_r + g*h*alpha_r + g*mlp_r (alpha = 1 on rank 0, 0 on rank 1),
 - host sums Z_0 + Z_1 per pair and scatters into hidden_states.

MLP runs in fp8e4 (DoubleRow perf mode, 2x PE throughput): gate/up/down
weights are pre-scaled powers of two chosen to keep fp8 operands in normal
range, activations (n2, act) are quantized on the fly, and the combined
scale is folded into the host-side gating multiplier. MLP weights are
DMA'd once into SBUF (3 large transfers issued at kernel start, landing
during the attention phase) and reused for both query halves.

Softmax normalization uses the scalar engine's Reciprocal activation read
directly from the PSUM row-sum row (partition 64, a legal PE base) plus a
1xN bf16 broadcast matmul; rmsnorm2 uses Rsqrt the same way. Both avoid
the slow DVE reciprocal and SBUF->SBUF descriptor hops.

Everything on-device runs in a transposed layout ([feature, token]);
rotate_half for rope is a PE matmul with a signed permutation matrix.
"""

import sys

sys.path.insert(0, "/opt/trn_rl_repo")

import numpy as np
import ml_dtypes

import concourse.bass as bass
import concourse.tile as tile
from concourse import mybir
from concourse import bass_utils as _bu
from concourse.bass_utils import run_bass_kernel_spmd
from concourse.vector_clock import ScopedClock, VectorClock



BF16 = mybir.dt.bfloat16
F32 = mybir.dt.float32
FP8 = mybir.dt.float8e4
AF = mybir.ActivationFunctionType
OP = mybir.AluOpType
PM = mybir.MatmulPerfMode

B, T, D = 4, 2048, 1024
H, KV, HD = 16, 8, 64
DFF = 4096
KSEL = 512
EPS = 1e-6

NQ = 512          # selected queries per batch (all of them, head-split)
NQH = 256         # query half processed per attention pass
ND = D // 128     # 8 d-tiles
NT = T // 128     # 16 key tiles
HL = H // 2       # 8 local q heads
KVL = KV // 2     # 4 local kv heads
NKC = KVL * HD // 128  # 2 local k-output chunks (2 kv heads each)
NQC = HL * HD // 128   # 4 local q-output chunks (2 q heads each)
NFC = DFF // 128       # 32 ff chunks
NFL = NFC // 2         # 16 local ff chunks (d_ff tensor-parallel)
NCORES = 8
PAIRS = [[0, 1], [2, 3], [4, 5], [6, 7]]

# fp8 weight scales (powers of two; folded into host-side gate multiplier)
S_G = 512.0
S_U = 16.0
S_D = 512.0

# local q-head layout: q-chunk 2c holds local heads (4c, 4c+2) on partition
# halves (local kv heads 2c / 2c+1), chunk 2c+1 holds (4c+1, 4c+3).
TILE_HEADS_L = []
for c in range(2):
    TILE_HEADS_L.append((4 * c, 4 * c + 2))
    TILE_HEADS_L.append((4 * c + 1, 4 * c + 3))
HEAD_PERM_L = np.array(
    [h * HD + i for pair in TILE_HEADS_L for h in pair for i in range(HD)])


# ---------------------------------------------------------------------------
# walrus workarounds: this toolchain encodes at most ONE semaphore wait per
# instruction. Split the tile tail-drain into per-proc drains and move excess
# waits onto NoOps.
# ---------------------------------------------------------------------------

def _patched_drain_and_barrier(self, tick_clock, wait_clock):
    gc = tick_clock.global_clock
    n = len(gc)
    for i in range(n):
        t = gc[i]
        if t > 0:
            vec = [0] * n
            vec[i] = t
            d = self.nc.sync.drain()
            wait_clock.add_sem_waits(d.ins, ScopedClock({None: VectorClock(vec)}))
    self.nc.all_engine_barrier()
    popped = self.nc._tile_sem_poison_stack.pop()
    assert popped is self._sem_poison
    self.nc.clear_and_free_semaphores(list(self.sems.allocated().values()))


tile.TileContext._drain_and_barrier = _patched_drain_and_barrier

_MAX_WAITS = 1


def _split_excess_waits(nc):
    for f in nc.m.functions:
        for bb in f.blocks:
            new = []
            for inst in bb.instructions:
                si = inst.sync_info
                if si is not None and si.on_wait is not None and len(si.on_wait) > _MAX_WAITS:
                    waits = list(si.on_wait)
                    excess, keep = waits[:-_MAX_WAITS], waits[-_MAX_WAITS:]
                    k = 0
                    while excess:
                        chunk, excess = excess[:_MAX_WAITS], excess[_MAX_WAITS:]
                        new.append(mybir.InstNoOp(
                            name=f"{inst.name}_ws{k}",
                            engine=inst.engine,
                            sync_info=mybir.SyncInfo(on_wait=chunk, on_update=[])))
                        k += 1
                    inst.sync_info = mybir.SyncInfo(
                        on_wait=keep, on_update=list(si.on_update or []))
                new.append(inst)
            bb.instructions = new


def act_unchecked(eng, out, in_, func, bias=0.0, scale=1.0):
    """scalar.activation without the Reciprocal/Rsqrt accuracy guard (our
    tolerance is 2e-2; the LUT error is ~1e-3)."""
    inputs = [eng.lower_ap(in_)]
    for arg in [bias, scale, 0.0]:
        if isinstance(arg, bass.AP):
            inputs.append(eng.lower_ap(arg))
        else:
            inputs.append(mybir.ImmediateValue(dtype=mybir.dt.float32, value=arg))
    outputs = [eng.lower_ap(out)]
    return eng.add_instruction(
        mybir.InstActivation(
            name=eng.bass.get_next_instruction_name(),
            func=func, ins=inputs, outs=outputs))


# ---------------------------------------------------------------------------
# device program
# ---------------------------------------------------------------------------

def build_program(qlo, qhi, dbg=False):
    """qlo/qhi: dict[(qh, tt)] compile-time query ranges within each 256-query
    half (uniform across cores/batches)."""
    nc = bass.Bass(trn_type="TRN2", target_bir_lowering=False, debug=False)

    def inp(name, shape, dt):
        return nc.dram_tensor(name, shape, dt, kind="ExternalInput").ap()

    # ALL inputs are host-pre-arranged partition-major so every DMA is a
    # linear copy (128 descriptors of 4-32KB); strided/transposing DMAs are
    # descriptor-rate bound (~8.5ns/descriptor) and 6-8x slower.
    xnC = inp("xnC", [4, 128, ND, 512], BF16)     # normalized hidden.T, chunked
    nselT = inp("nselT", [128, ND, NQ], BF16)     # host-normalized selected.T
    selO = inp("selO", [128, ND, NQ], BF16)       # raw selected rows.T
    qwT = inp("qwT", [128, ND, HL * HD], BF16)
    kwT = inp("kwT", [128, ND, KVL * HD], BF16)
    vwT = inp("vwT", [128, ND, KVL * HD], BF16)
    owT = inp("owT", [128, NQC, D], BF16)
    gw = inp("gw", [128, NFL, ND, 128], FP8)
    uw = inp("uw", [128, NFL, ND, 128], FP8)
    dw = inp("dw", [128, ND, NFL, 128], FP8)
    # packed small constants: f32 block and bf16 block (one DMA each)
    NCF = 4 + 2 + KVL * HD + NQ + NT + NQ + NQ
    NCB = 128 + NQ + NQ + 64 + 2 * T
    cstF = inp("cstF", [128, NCF], F32)
    cstB = inp("cstB", [128, NCB], BF16)
    selg = inp("selg", [128, ND, NQ], F32)  # selres*(1-g) on rank 0, zeros rank 1

    updT = nc.dram_tensor("updT", [128, ND, NQ], BF16,
                          kind="ExternalOutput").ap()

    with tile.TileContext(nc, pool_alloc_mode="queue") as tc:
        with tc.tile_pool(name="ps", bufs=8, space="PSUM") as ps, \
             tc.tile_pool(name="persist", bufs=1) as pp, \
             tc.tile_pool(name="rows", bufs=2) as rowp, \
             tc.tile_pool(name="dramp", bufs=1, space="DRAM") as dram:

            # AllGather of fp8 partials + local add beats AllReduce: the CC
            # cost is a fixed ~15us overhead plus bytes moved at ~30GB/s, and
            # AR pays a 1.875x protocol multiplier on top.
            cc_in = [dram.tile([128, ND, NQH], FP8, name=f"cc_in{i}")
                     for i in range(2)]
            cc_out = [dram.tile([2, 128, ND, NQH], FP8, name=f"cc_out{i}")
                      for i in range(2)]

            # ---- persistent tiles ------------------------------------------
            hTt = pp.tile([128, ND, NQ], BF16, name="hTt")
            n2T = pp.tile([128, ND, NQ], FP8, name="n2T")
            ctxT = pp.tile([128, NQC, NQ], BF16, name="ctxT")
            actT = pp.tile([128, NFL, NQ], FP8, name="actT")
            ones_t = pp.tile([128, 1], BF16, name="ones_t")
            nc.vector.memset(ones_t, 1.0)
            eps_t = pp.tile([1, 1], F32, name="eps_t")
            nc.vector.memset(eps_t, EPS)
            ones65 = pp.tile([65, 128], BF16, name="ones65")
            nc.vector.memset(ones65, 1.0)

            cF = pp.tile([128, NCF], F32, name="cF")
            cB = pp.tile([128, NCB], BF16, name="cB")
            o_ = 0
            c_qb = cF[:, o_:o_ + NQC]; o_ += NQC
            c_kb = cF[:, o_:o_ + NKC]; o_ += NKC
            c_vb = cF[:, o_:o_ + KVL * HD]; o_ += KVL * HD
            c_pos = cF[:, o_:o_ + NQ]; o_ += NQ
            c_tv = cF[:, o_:o_ + NT]; o_ += NT
            c_g = cF[:, o_:o_ + NQ]; o_ += NQ
            c_gh = cF[:, o_:o_ + NQ]; o_ += NQ
            assert o_ == NCF
            c_rm = cB[:, 0:128]
            c_cq = cB[:, 128:128 + NQ]
            c_sq = cB[:, 128 + NQ:128 + 2 * NQ]
            ident64 = cB[:, 128 + 2 * NQ:128 + 2 * NQ + 64]  # I on parts 0:64
            _o2 = 128 + 2 * NQ + 64
            c_ck = cB[:, _o2:_o2 + T]
            c_sk = cB[:, _o2 + T:_o2 + 2 * T]
            pA_cm = tc.tile_pool(name="pA", bufs=1)
            pA = pA_cm.__enter__()
            kT = pA.tile([128, NKC, T], BF16, name="kT")
            vplus = pA.tile([128, NT, KVL, HD + 1], BF16, name="vplus")
            nc.vector.memset(vplus[:, :, :, HD:HD + 1], 1.0)
            qrT = pA.tile([128, NQC, NQ], BF16, name="qrT")
            w_o = pA.tile([128, NQC, D], BF16, name="w_o")
            selOs = pA.tile([128, ND, NQ], BF16, name="selOs")

            p4_cm = tc.tile_pool(name="ph4", bufs=1)
            p4 = p4_cm.__enter__()

            pN_cm = tc.tile_pool(name="pN", bufs=1)
            pN = pN_cm.__enter__()
            xn = pN.tile([128, 4, ND, 512], BF16, name="xn")
            w_k = pN.tile([128, ND, KVL * HD], BF16, name="w_k")
            w_v = pN.tile([128, ND, KVL * HD], BF16, name="w_v")
            w_q = pN.tile([128, ND, HL * HD], BF16, name="w_q")
            nsel = pN.tile([128, ND, NQ], BF16, name="nsel")

            # ---- input DMAs (all linear; issue order = priority per engine;
            # xn chunks spread over the three DMA queues) ---
            nc.gpsimd.dma_start(out=w_k[:, :, 0:128], in_=kwT[:, :, 0:128])
            nc.gpsimd.dma_start(out=w_k[:, :, 128:256], in_=kwT[:, :, 128:256])
            nc.gpsimd.dma_start(out=w_v, in_=vwT)
            nc.sync.dma_start(out=xn[:, 0, 0:ND // 2], in_=xnC[0][:, 0:ND // 2])
            nc.sync.dma_start(out=xn[:, 0, ND // 2:ND],
                              in_=xnC[0][:, ND // 2:ND])
            nc.sync.dma_start(out=xn[:, 3], in_=xnC[3])
            nc.scalar.dma_start(out=cF, in_=cstF)
            nc.scalar.dma_start(out=cB, in_=cstB)
            nc.scalar.dma_start(out=xn[:, 1], in_=xnC[1])
            nc.gpsimd.dma_start(out=nsel, in_=nselT)
            nc.gpsimd.dma_start(out=w_q, in_=qwT)
            nc.gpsimd.dma_start(out=xn[:, 2], in_=xnC[2])
            nc.gpsimd.dma_start(out=w_o, in_=owT)
            nc.gpsimd.dma_start(out=selOs, in_=selO)

            # ==================================================================
            # Phase 1: K (+rope) and V per 512-token chunk, interleaved with
            # the first attention t-loop to hide the xn DMA stream.
            # ==================================================================
            def kv_chunk(ch, p2):
                    cs = slice(ch * 512, (ch + 1) * 512)
                    for kc in range(NKC):
                        kps = ps.tile([128, 512], F32, name="kps", tag="ps")
                        for dt in range(ND):
                            nc.tensor.matmul(
                                kps, lhsT=w_k[:, dt, kc * 128:(kc + 1) * 128],
                                rhs=xn[:, ch, dt, :],
                                start=(dt == 0), stop=(dt == ND - 1))
                        kraw = p2.tile([128, 512], BF16, name="kraw")
                        nc.vector.tensor_scalar(
                            out=kraw, in0=kps, scalar1=c_kb[:, kc:kc + 1],
                            scalar2=None, op0=OP.add)
                        rot = ps.tile([128, 512], F32, name="rot", tag="ps")
                        nc.tensor.matmul(rot, lhsT=c_rm, rhs=kraw,
                                         start=True, stop=True)
                        dst = kT[:, kc, cs]
                        tmp = p2.tile([128, 512], BF16, name="tmp")
                        nc.vector.tensor_mul(out=tmp, in0=rot, in1=c_sk[:, cs])
                        nc.vector.tensor_mul(out=dst, in0=kraw, in1=c_ck[:, cs])
                        nc.vector.tensor_add(out=dst, in0=dst, in1=tmp)

                    for tt in range(ch * 4, ch * 4 + 4):
                        vps = ps.tile([128, 512], F32, name="vps", tag="ps")
                        to = (tt % 4) * 128
                        for dt in range(ND):
                            nc.tensor.matmul(
                                vps[:, 0:KVL * HD],
                                lhsT=xn[:, ch, dt, to:to + 128],
                                rhs=w_v[:, dt, :],
                                start=(dt == 0), stop=(dt == ND - 1))
                        nc.vector.tensor_add(
                            out=vplus[:, tt, :, 0:HD],
                            in0=vps[:, 0:KVL * HD].rearrange(
                                "p (h d) -> p h d", h=KVL),
                            in1=c_vb.rearrange("p (h d) -> p h d", h=KVL))

            # ==================================================================
            # Phase 2: Q proj + rope (host-normalized input)
            # ==================================================================
            def qproj(p3):
                for qc in range(NQC):
                    qps = ps.tile([128, 512], F32, name="qps", tag="ps")
                    for dt in range(ND):
                        nc.tensor.matmul(
                            qps[:, 0:NQ], lhsT=w_q[:, dt, qc * 128:(qc + 1) * 128],
                            rhs=nsel[:, dt, :],
                            start=(dt == 0), stop=(dt == ND - 1))
                    qraw = p3.tile([128, NQ], BF16, name="qraw")
                    nc.vector.tensor_scalar(
                        out=qraw, in0=qps[:, 0:NQ], scalar1=c_qb[:, qc:qc + 1],
                        scalar2=None, op0=OP.add)
                    rotq = ps.tile([128, 512], F32, name="rotq", tag="ps")
                    nc.tensor.matmul(rotq[:, 0:NQ], lhsT=c_rm, rhs=qraw,
                                     start=True, stop=True)
                    dst = qrT[:, qc, :]
                    tmpq = p3.tile([128, NQ], BF16, name="tmpq")
                    nc.vector.tensor_mul(out=tmpq, in0=rotq[:, 0:NQ], in1=c_sq)
                    nc.vector.tensor_mul(out=dst, in0=qraw, in1=c_cq)
                    nc.vector.tensor_add(out=dst, in0=dst, in1=tmpq)

            # ==================================================================
            # Phases 3-7 per query half: attention t-loop, eviction, o-proj +
            # AllReduce (overlapped), h + rmsnorm2, d_ff-split MLP + Z output.
            # ==================================================================

            def alloc_cps(qh):
                # one accumulator per (kc, kv-half); free dim packs BOTH
                # q-head (ab) blocks so the ctx matmul is a single 3D-out
                # instruction per (tt, kc, half)
                cps = {}
                for kc in range(NKC):
                    for hf in range(2):
                        cps[(kc, hf)] = ps.tile([128, 512], F32,
                                                name=f"cps{qh}{kc}{hf}",
                                                tag="ps")
                return cps

            def attn_tloop(qh, cps, tts=None, hooks=None):
                qs0 = qh * NQH
                live = [t_ for t_ in range(NT) if qlo[(qh, t_)] < NQH]
                last_tt = max(live)
                for ti, tt in enumerate(live if tts is None else tts):
                    if hooks is not None and ti in hooks:
                        hooks[ti]()
                    lo = qlo[(qh, tt)]
                    hi = qhi[(qh, tt)]
                    mask = None
                    if hi > lo:
                        mask = p4.tile([128, 512], BF16, name="mask", bufs=2)
                        for mh in range(2):
                            nc.vector.tensor_scalar(
                                out=mask[:, mh * NQH + lo:mh * NQH + hi],
                                in0=c_pos[:, qs0 + lo:qs0 + hi],
                                scalar1=c_tv[:, tt:tt + 1], scalar2=None,
                                op0=OP.is_ge)
                    # stage-major within the t-tile: all scores first, then
                    # all ctx matmuls — the exps pipeline on the scalar
                    # engine behind the remaining scores instead of
                    # serializing a score->exp->ctx ring per head group.
                    ptl = []
                    for kc in range(NKC):
                        for half in range(2):
                            hs_ = slice(half * 64, (half + 1) * 64)
                            sp = ps.tile([128, 512], F32, name="sp", tag="ps")
                            # one matmul streams both q-head chunks (3D rhs)
                            nc.tensor.matmul(
                                sp.rearrange("p (a q) -> p a q", a=2)[:, :, lo:NQH],
                                lhsT=kT[hs_, kc, tt * 128:(tt + 1) * 128],
                                rhs=qrT[hs_, 2 * kc:2 * kc + 2,
                                        qs0 + lo:qs0 + NQH],
                                start=True, stop=True)
                            pt = p4.tile([128, 2, NQH], BF16, name="pt", bufs=8)
                            nc.scalar.activation(
                                out=pt[:, :, lo:NQH],
                                in_=sp.rearrange("p (h q) -> p h q", h=2)[:, :, lo:NQH],
                                func=AF.Exp)
                            if mask is not None:
                                nc.vector.tensor_mul(
                                    out=pt[:, :, lo:hi],
                                    in0=pt[:, :, lo:hi],
                                    in1=mask.rearrange(
                                        "p (h q) -> p h q", h=2)[:, :, lo:hi])
                            ptl.append((kc, half, pt))
                    for kc, half, pt in ptl:
                        cp = cps[(kc, half)]
                        nc.tensor.matmul(
                            cp.rearrange("p (a q) -> p a q",
                                         a=2)[0:HD + 1, :, lo:NQH],
                            lhsT=vplus[:, tt, 2 * kc + half, 0:HD + 1],
                            rhs=pt[:, :, lo:NQH],
                            start=(tt == 0), stop=(tt == last_tt))

            def attn_evict(qh, cps):
                """ctx rows sit at PSUM partitions 0..63, the row-sum at
                partition 64 (a legal PE base). Reciprocal on the scalar
                engine straight from PSUM, 1x64 bf16 broadcast matmul, then
                per-head scaling. Second head of each pair staged and moved
                with a single SBUF->SBUF DMA."""
                qsl = slice(qh * NQH, qh * NQH + NQH)
                tiles = [(k_, h_) for k_ in range(NKC) for h_ in range(2)]
                # stage-major emission: all recips, then all broadcasts, ...
                # so the four tiles pipeline across engines instead of
                # serializing one ~1.7us chain per tile.
                rrs, rbs, casts, stages, psbs = [], [], {}, [], []
                for kc, hf in tiles:
                    cp = cps[(kc, hf)]
                    rr = p4.tile([65, 512], BF16, name="rr", bufs=4)
                    act_unchecked(nc.scalar, rr[64:65, :], cp[64:65, :],
                                  AF.Reciprocal)
                    rrs.append(rr)
                for g, (kc, hf) in enumerate(tiles):
                    rb = ps.tile([128, 512], F32, name="rb", tag="ps")
                    nc.tensor.matmul(rb[0:64, :], lhsT=ones65[64:65, 0:64],
                                     rhs=rrs[g][64:65, :],
                                     start=True, stop=True)
                    rbs.append(rb)
                for g, (kc, hf) in enumerate(tiles):
                    rb_sb = p4.tile([64, 512], BF16, name="rb_sb", bufs=4)
                    nc.vector.tensor_copy(out=rb_sb, in_=rbs[g][0:64, :])
                    casts[(kc, hf)] = rb_sb
                # output chunk 2kc+ab: head[0] from kv-half-0 tile's ab block,
                # head[1] from kv-half-1 tile's ab block
                chunks = [(k_, a_) for k_ in range(NKC) for a_ in range(2)]
                for kc, ab in chunks:
                    absl = slice(ab * NQH, (ab + 1) * NQH)
                    # second head of the pair staged for the PE identity move
                    # (SBUF->SBUF DMAs are descriptor-rate bound: ~17us)
                    stage = p4.tile([64, NQH], BF16, name="stage", bufs=4)
                    nc.vector.tensor_mul(
                        out=stage,
                        in0=cps[(kc, 1)][0:HD, absl],
                        in1=casts[(kc, 1)][:, absl])
                    stages.append(stage)
                    nc.vector.tensor_mul(
                        out=ctxT[0:64, 2 * kc + ab, qsl],
                        in0=cps[(kc, 0)][0:HD, absl],
                        in1=casts[(kc, 0)][:, absl])
                for g, (kc, ab) in enumerate(chunks):
                    psb = ps.tile([128, 512], F32, name="psb", tag="ps")
                    nc.tensor.matmul(psb[64:128, 0:NQH],
                                     lhsT=ident64[0:64, :], rhs=stages[g],
                                     start=True, stop=True)
                    psbs.append(psb)
                for g, (kc, ab) in enumerate(chunks):
                    nc.vector.tensor_copy(
                        out=ctxT[64:128, 2 * kc + ab, qsl],
                        in_=psbs[g][64:128, 0:NQH])

            def oproj(qh, p5):
                qsl = slice(qh * NQH, qh * NQH + NQH)
                o_st = p5.tile([128, ND, NQH], FP8, name="o_st")
                for dc in range(ND):
                    ops_ = ps.tile([128, 512], F32, name="ops_", tag="ps")
                    for hc in range(NQC):
                        nc.tensor.matmul(
                            ops_[:, 0:NQH],
                            lhsT=w_o[:, hc, dc * 128:(dc + 1) * 128],
                            rhs=ctxT[:, hc, qsl],
                            start=(hc == 0), stop=(hc == NQC - 1))
                    nc.vector.tensor_copy(out=o_st[:, dc, :], in_=ops_[:, 0:NQH])
                nc.sync.dma_start(out=cc_in[qh], in_=o_st)
                nc.gpsimd.collective_compute(
                    "AllGather", OP.bypass, replica_groups=PAIRS,
                    ins=[cc_in[qh].opt()], outs=[cc_out[qh].opt()])

            def hnorm(qh, p6, selg_s):
                """h = AR + residual for this half; rmsnorm2 -> n2T half (fp8);
                Z base ghs = selg + c_gh * h."""
                qsl = slice(qh * NQH, qh * NQH + NQH)
                hsb = p6.tile([128, 2, ND, NQH], FP8, name="hsb")
                nc.sync.dma_start(out=hsb[:, 0], in_=cc_out[qh][0])
                nc.sync.dma_start(out=hsb[:, 1], in_=cc_out[qh][1])
                ssn = ps.tile([128, 512], F32, name="ssn", tag="ps")
                for dt in range(ND):
                    eng = nc.vector if dt % 2 == 0 else nc.gpsimd
                    eng.tensor_add(out=hTt[:, dt, qsl],
                                   in0=hsb[:, 0, dt, :],
                                   in1=hsb[:, 1, dt, :])
                    eng.tensor_add(out=hTt[:, dt, qsl],
                                   in0=hTt[:, dt, qsl],
                                   in1=selOs[:, dt, qsl])
                    sq6 = p6.tile([128, NQH], BF16, name="sq6", bufs=4)
                    eng.tensor_mul(out=sq6, in0=hTt[:, dt, qsl],
                                   in1=hTt[:, dt, qsl])
                    nc.tensor.matmul(ssn[0:1, 0:NQH], lhsT=ones_t, rhs=sq6,
                                     start=(dt == 0), stop=(dt == ND - 1))
                rrow = rowp.tile([1, NQH], BF16, name="rrow", tag="row")
                act_unchecked(nc.scalar, rrow, ssn[0:1, 0:NQH], AF.Rsqrt,
                              bias=eps_t[0:1, 0:1], scale=1.0 / D)
                rbc = ps.tile([128, 512], F32, name="rbc", tag="ps")
                nc.tensor.matmul(rbc[:, 0:NQH], lhsT=ones65[0:1, :], rhs=rrow,
                                 start=True, stop=True)
                rbc_sb = p6.tile([128, NQH], BF16, name="rbc_sb")
                nc.vector.tensor_copy(out=rbc_sb, in_=rbc[:, 0:NQH])
                rbc_b4 = bass.AP(tensor=rbc_sb.tensor, offset=rbc_sb.offset,
                                 ap=[rbc_sb.ap[0], [0, ND // 2], rbc_sb.ap[1]])
                nc.vector.tensor_mul(out=n2T[:, 0:ND // 2, qsl],
                                     in0=hTt[:, 0:ND // 2, qsl], in1=rbc_b4)
                nc.gpsimd.tensor_mul(out=n2T[:, ND // 2:ND, qsl],
                                     in0=hTt[:, ND // 2:ND, qsl], in1=rbc_b4)

            def hnorm_gh(qh, p6, selg_s):
                """Z base update ghs = selg + c_gh * h; gpsimd-only, emitted
                after the collectives so it does not delay their triggers."""
                qsl = slice(qh * NQH, qh * NQH + NQH)
                for dt in range(ND):
                    gh_t = p6.tile([128, NQH], F32, name="gh_t")
                    nc.gpsimd.tensor_mul(out=gh_t, in0=hTt[:, dt, qsl],
                                         in1=c_gh[:, qsl])
                    nc.gpsimd.tensor_add(out=selg_s[:, dt, qsl], in0=gh_t,
                                         in1=selg_s[:, dt, qsl])

            def mlp_gateup(qh, p7, fc_lo, fc_hi, hooks=None):
                qsl = slice(qh * NQH, qh * NQH + NQH)
                for fc in range(fc_lo, fc_hi):
                    if hooks is not None and fc in hooks:
                        hooks[fc]()
                    gps = ps.tile([128, 512], F32, name="gps", tag="ps")
                    ups = ps.tile([128, 512], F32, name="ups", tag="ps")
                    for kk in range(ND // 2):
                        nc.tensor.matmul(
                            gps[:, 0:NQH], lhsT=w_g[:, fc, 2 * kk:2 * kk + 2, :],
                            rhs=n2T[:, 2 * kk:2 * kk + 2, qsl],
                            start=(kk == 0), stop=(kk == ND // 2 - 1),
                            perf_mode=PM.DoubleRow)
                    for kk in range(ND // 2):
                        nc.tensor.matmul(
                            ups[:, 0:NQH], lhsT=w_u[:, fc, 2 * kk:2 * kk + 2, :],
                            rhs=n2T[:, 2 * kk:2 * kk + 2, qsl],
                            start=(kk == 0), stop=(kk == ND // 2 - 1),
                            perf_mode=PM.DoubleRow)
                    sg = p7.tile([128, NQH], BF16, name="sg", bufs=4)
                    nc.scalar.activation(out=sg, in_=gps[:, 0:NQH], func=AF.Silu,
                                         scale=1.0 / S_G)
                    nc.vector.tensor_mul(out=actT[:, fc, qsl],
                                         in0=ups[:, 0:NQH], in1=sg)

            def mlp_down(qh, p7, selg_s):
                qsl = slice(qh * NQH, qh * NQH + NQH)
                for dc in range(ND):
                    mps = ps.tile([128, 512], F32, name="mps", tag="ps")
                    for kk in range(NFL // 2):
                        nc.tensor.matmul(
                            mps[:, 0:NQH], lhsT=w_d[:, dc, 2 * kk:2 * kk + 2, :],
                            rhs=actT[:, 2 * kk:2 * kk + 2, qsl],
                            start=(kk == 0), stop=(kk == NFL // 2 - 1),
                            perf_mode=PM.DoubleRow)
                    f1 = p7.tile([128, NQH], BF16, name="f1", bufs=8)
                    nc.vector.tensor_mul(out=f1, in0=mps[:, 0:NQH],
                                         in1=c_g[:, qsl])
                    nc.vector.tensor_add(out=f1, in0=f1,
                                         in1=selg_s[:, dc, qsl])
                    (nc.gpsimd if dc % 2 == 0 else nc.scalar).dma_start(
                        out=updT[:, dc, qsl], in_=f1)

            # interleave kv chunks with the first attention pass
            with tc.tile_pool(name="ph2", bufs=3) as p2, \
                 tc.tile_pool(name="ph3", bufs=3) as p3:
                cps0 = alloc_cps(0)
                live0 = [t_ for t_ in range(NT) if qlo[(0, t_)] < NQH]
                kv_chunk(0, p2)
                qproj(p3)
                kv_chunk(1, p2)
                attn_tloop(0, cps0, tts=[t_ for t_ in live0 if t_ < 4])
                kv_chunk(2, p2)
                attn_tloop(0, cps0, tts=[t_ for t_ in live0 if 4 <= t_ < 8])
                kv_chunk(3, p2)
                attn_tloop(0, cps0, tts=[t_ for t_ in live0 if t_ >= 8])
                attn_evict(0, cps0)

            pN_cm.__exit__(None, None, None)

            # MLP weights + gating state live in the space freed by pN; the
            # 3 big fp8 transfers stream in under attention pass 1.
            pB_cm = tc.tile_pool(name="pB", bufs=1)
            pB = pB_cm.__enter__()
            w_g = pB.tile([128, NFL, ND, 128], FP8, name="w_g")
            w_u = pB.tile([128, NFL, ND, 128], FP8, name="w_u")
            w_d = pB.tile([128, ND, NFL, 128], FP8, name="w_d")
            selg_s = pB.tile([128, ND, NQ], F32, name="selg_s")
            def load_mlp_w():
                # issued from inside attention pass 1: their SBUF region
                # reuses xn's (so the transfers cannot start before the kv
                # phase drains anyway), and the scalar queue is clear of
                # latency-critical DMAs from here to the end of the pass.
                nc.scalar.dma_start(out=selg_s, in_=selg)
                nc.scalar.dma_start(out=w_g, in_=gw)
                nc.scalar.dma_start(out=w_u, in_=uw)
                nc.scalar.dma_start(out=w_d, in_=dw)

            with tc.tile_pool(name="ph5", bufs=1) as p5, \
                 tc.tile_pool(name="ph6", bufs=2) as p6, \
                 tc.tile_pool(name="ph7", bufs=2) as p7:
                cps1 = alloc_cps(1)
                attn_tloop(1, cps1,
                           hooks={0: lambda: oproj(0, p5),
                                  14: load_mlp_w})
                attn_evict(1, cps1)
                oproj(1, p5)
                hnorm(0, p6, selg_s)
                hnorm_gh(0, p6, selg_s)
                mlp_gateup(0, p7, 0, NFL,
                           hooks={10: lambda: hnorm(1, p6, selg_s)})
                mlp_down(0, p7, selg_s)
                hnorm_gh(1, p6, selg_s)
                mlp_gateup(1, p7, 0, NFL)
                mlp_down(1, p7, selg_s)

            pB_cm.__exit__(None, None, None)
            p4_cm.__exit__(None, None, None)
            pA_cm.__exit__(None, None, None)

    _split_excess_waits(nc)
    return nc


# ---------------------------------------------------------------------------
# host side
# ---------------------------------------------------------------------------

def _bf16(x):
    return np.asarray(x, dtype=np.float32).astype(ml_dtypes.bfloat16)


def _fp8(x):
    return np.asarray(x, dtype=np.float32).astype(ml_dtypes.float8_e4m3fn)


def _rope_matrix():
    """R[k, p] = sign(p) * 1[k == swap(p)]; (R.T @ x)[p] = sign(p)*x[swap(p)]."""
    R = np.zeros((128, 128), np.float32)
    for p in range(128):
        base = (p // 64) * 64
        off = p % 64
        if off < 32:
            R[base + off + 32, p] = -1.0
        else:
            R[base + off - 32, p] = 1.0
    return R


def _install_ntff_hook():
    """Shim antenv.axon_hooks (absent in this image) so trace=True works."""
    import types
    try:
        import antenv.axon_hooks  # noqa: F401
        return
    except ImportError:
        pass
    try:
        from trn_agent_boot.trn_boot import _ntff_profile_via_ctypes
        hook = _ntff_profile_via_ctypes("/opt/axon/libaxon_pjrt.so")
    except Exception:
        hook = None
    mod = types.ModuleType("antenv.axon_hooks")
    mod._hook = hook
    mod.set_axon_ntff_profile_hook = lambda h: setattr(mod, "_hook", h)
    mod.get_axon_ntff_profile_hook = lambda: mod._hook
    sys.modules["antenv.axon_hooks"] = mod


def kernel(hidden_states, token_indices, batch_indices, gating_scores, cos, sin,
           ln1_w, ln2_w, q_w, q_b, k_w, k_b, v_w, v_b, o_w, gate_w, up_w, down_w,
           _profile=False, _dbg=False):
    hidden_states = np.asarray(hidden_states, dtype=np.float32)
    token_indices = np.asarray(token_indices).astype(np.int64)
    gating_scores = np.asarray(gating_scores, dtype=np.float32)
    cos = np.asarray(cos, dtype=np.float32)
    sin = np.asarray(sin, dtype=np.float32)
    ln1_w = np.asarray(ln1_w, dtype=np.float32)
    ln2_w = np.asarray(ln2_w, dtype=np.float32)

    topk = token_indices.reshape(B, KSEL)
    gsc = gating_scores.reshape(B, KSEL)

    qlo, qhi = {}, {}
    for qh in range(2):
        for tt in range(NT):
            los, his = [], []
            for b in range(B):
                pos_q = np.asarray(topk[b, qh * NQH:(qh + 1) * NQH])
                los.append(int(np.searchsorted(pos_q, tt * 128)))
                his.append(int(np.searchsorted(pos_q, tt * 128 + 126,
                                               side="right")))
            qlo[(qh, tt)] = min(los)
            qhi[(qh, tt)] = max(his)

    nc = build_program(qlo, qhi, dbg=_dbg)

    q_w_eff = (np.asarray(q_w, np.float32) * ln1_w[None, :]) / 8.0
    k_w_eff = np.asarray(k_w, np.float32) * ln1_w[None, :]
    v_w_eff = np.asarray(v_w, np.float32) * ln1_w[None, :]
    g_w_eff = np.asarray(gate_w, np.float32) * ln2_w[None, :] * S_G
    u_w_eff = np.asarray(up_w, np.float32) * ln2_w[None, :] * S_U
    q_b_eff = np.asarray(q_b, np.float32) / 8.0
    down_f = np.asarray(down_w, np.float32) * S_D

    tvals = (np.arange(NT)[None, :] * 128 + np.arange(128)[:, None]).astype(np.float32)
    rope_m = _rope_matrix()

    def pmaj(a):
        """[c, 128, x] -> [128, c, x] partition-major."""
        return np.ascontiguousarray(a.transpose(1, 0, 2))

    # per-half shards: attention heads AND d_ff halves keyed by rank hh
    half_w = []
    for hh in range(2):
        qsl = slice(hh * HL * HD, (hh + 1) * HL * HD)
        ksl = slice(hh * KVL * HD, (hh + 1) * KVL * HD)
        fsl = slice(hh * (DFF // 2), (hh + 1) * (DFF // 2))
        qwT = _bf16(pmaj(q_w_eff.T[:, qsl][:, HEAD_PERM_L]
                         .reshape(ND, 128, HL * HD)))
        kwT = _bf16(pmaj(k_w_eff.T[:, ksl].reshape(ND, 128, KVL * HD)))
        vwT = _bf16(pmaj(v_w_eff.T[:, ksl].reshape(ND, 128, KVL * HD)))
        owT = _bf16(pmaj(np.asarray(o_w, np.float32).T[qsl, :][HEAD_PERM_L, :]
                         .reshape(NQC, 128, D)))
        qb_a = np.ascontiguousarray(
            q_b_eff[qsl][HEAD_PERM_L].reshape(NQC, 128).T).astype(np.float32)
        kb_a = np.ascontiguousarray(
            np.asarray(k_b, np.float32)[ksl].reshape(NKC, 128).T)
        vb_a = np.broadcast_to(np.asarray(v_b, np.float32)[ksl][None, :],
                               (128, KVL * HD))
        gwa = _fp8(np.ascontiguousarray(
            g_w_eff[fsl].reshape(NFL, 128, ND, 128).transpose(3, 0, 2, 1)))
        uwa = _fp8(np.ascontiguousarray(
            u_w_eff[fsl].reshape(NFL, 128, ND, 128).transpose(3, 0, 2, 1)))
        dwa = _fp8(np.ascontiguousarray(
            down_f[:, fsl].reshape(ND, 128, NFL, 128).transpose(3, 0, 2, 1)))
        half_w.append(dict(qwT=qwT, kwT=kwT, vwT=vwT, owT=owT,
                           gw=gwa, uw=uwa, dw=dwa,
                           _qb=qb_a, _kb=kb_a, _vb=vb_a))

    def stack2(mat):
        mT = mat.T.astype(np.float32)
        return np.concatenate([mT, mT], axis=0)

    def rms_rows(x):
        v = np.mean(x * x, axis=-1, keepdims=True)
        return x / np.sqrt(v + EPS)

    in_maps = []
    zeros_selg = np.zeros((128, ND, NQ), np.float32)
    zeros_gh = np.zeros((128, NQ), np.float32)
    for c in range(NCORES):
        b = c // 2
        hh = c % 2
        pos_all = np.asarray(topk[b], dtype=np.int64)
        g_all = gsc[b]
        sel = hidden_states[b][pos_all]
        xn_host = rms_rows(hidden_states[b]) * ln1_w
        nsel_host = rms_rows(sel) * ln1_w
        hw = half_w[hh]
        im = {k: v for k, v in hw.items() if not k.startswith("_")}
        g_bc = np.broadcast_to(g_all.astype(np.float32)[None, :], (128, NQ))
        posq = np.broadcast_to(pos_all.astype(np.float32)[None, :], (128, NQ))
        cstF = np.concatenate(
            [hw["_qb"], hw["_kb"], hw["_vb"], posq, tvals,
             g_bc / (S_U * S_D),
             g_bc if hh == 0 else zeros_gh], axis=1).astype(np.float32)
        ident64 = np.zeros((128, 64), np.float32)
        ident64[np.arange(64), np.arange(64)] = 1.0
        cstB = _bf16(np.concatenate(
            [rope_m, stack2(cos[b][pos_all]), stack2(sin[b][pos_all]),
             ident64, stack2(cos[b]), stack2(sin[b])], axis=1))
        im.update(
            xnC=_bf16(xn_host.T.reshape(ND, 128, 4, 512).transpose(2, 1, 0, 3)),
            nselT=_bf16(pmaj(nsel_host.T.reshape(ND, 128, NQ))),
            selO=_bf16(pmaj(sel.T.reshape(ND, 128, NQ))),
            cstF=np.ascontiguousarray(cstF),
            cstB=np.ascontiguousarray(cstB),
            selg=pmaj((sel * (1.0 - g_all)[:, None]).T.reshape(ND, 128, NQ)
                      ).astype(np.float32) if hh == 0 else zeros_selg,
        )
        in_maps.append(im)

    if _profile:
        _install_ntff_hook()
    res = run_bass_kernel_spmd(nc, in_maps, core_ids=list(range(NCORES)),
                               trace=_profile)

    out = hidden_states.copy()
    for pr in range(B):
        z0 = np.asarray(res.results[2 * pr]["updT"],
                        np.float32).transpose(1, 0, 2).reshape(D, NQ).T
        z1 = np.asarray(res.results[2 * pr + 1]["updT"],
                        np.float32).transpose(1, 0, 2).reshape(D, NQ).T
        out[pr, np.asarray(topk[pr]), :] = z0 + z1
    if _profile or _dbg:
        return out, res
    return out
